# revision 1
# baseline (speedup 1.0000x reference)
"""Trainium2 Bass kernel for nn_ABNet: 10-head MLP ensemble + per-sample QP.

Reference computation (per sample, all heads):
  h1  = relu(x @ W1[h] + b1[h])            x:[B,4]  -> [B,1024]
  x21 = relu(h1 @ W21[h] + b21[h])         -> [B,1024]
  x22 = relu(h1 @ W22[h] + b22[h])         -> [B,1024]
  x31 = x21 @ W31[h] + b31[h]              -> [B,2]
  x32 = 4*sigmoid(x22 @ W32[h] + b32[h])   -> [B,2]
  + closed-form single-constraint QP epilogue, softmax(wt) ensemble.

Strategy: pure data parallel over batch across 8 NeuronCores (B=32768 ->
4096/core).  Feature-major layout on chip (hidden dim on partitions, batch
on the free axis) so all matmuls use natural weight layouts with no
transposes.  Weights are host-cast to bf16 and streamed per head; the QP
epilogue runs batch-major after a PE transpose of the tiny per-head
outputs.  Everything is built with the Tile framework (auto scheduling).
"""

import numpy as np
import ml_dtypes

import concourse.bass as bass
import concourse.mybir as mybir
import concourse.tile as tile
from concourse.vector_clock import ScopedClock
from concourse.masks import make_identity
from concourse.bass_utils import run_bass_kernel_spmd

BF16 = mybir.dt.bfloat16
F32 = mybir.dt.float32
AF = mybir.ActivationFunctionType
OP = mybir.AluOpType

H, F_IN, H1, C = 10, 4, 1024, 2
KT = H1 // 128  # 8 k-tiles of the hidden dim
OBS_X, OBS_Y, RAD = 40.0, 15.0, 6.0
N_CORES = 8
B_FULL = 32768

_drain_patched = False


def _patch_tile_drain():
    """This container's walrus rejects >2 sync waits on one CTRL op; move the
    Tile kernel-tail drain waits onto individual SP NOPs."""
    global _drain_patched
    if _drain_patched:
        return
    _drain_patched = True

    def _drain_and_barrier(self, tick_clock, wait_clock):
        nc = self.nc
        carrier = nc.sync.nop()
        wait_clock.add_sem_waits(
            carrier.ins, ScopedClock({None: tick_clock.global_clock})
        )
        si = carrier.ins.sync_info
        waits = list(si.on_wait) if si and si.on_wait else []
        if len(waits) > 1:
            carrier.ins.sync_info = mybir.SyncInfo(on_wait=[waits[0]], on_update=[])
            for w in waits[1:]:
                nop = nc.sync.nop()
                nop.ins.sync_info = mybir.SyncInfo(on_wait=[w], on_update=[])
        nc.sync.drain()
        nc.all_engine_barrier()
        assert self.sems is not None
        popped = nc._tile_sem_poison_stack.pop()
        assert popped is self._sem_poison
        nc.clear_and_free_semaphores(list(self.sems.allocated().values()))
        nc.all_engine_barrier()

    tile.TileContext._drain_and_barrier = _drain_and_barrier


def _split_excess_waits(nc, max_waits=1):
    """This walrus build rejects instructions carrying more than a couple of
    semaphore waits; hoist the excess onto same-engine NoOps just before."""
    for fn in nc.m.functions:
        for bb in fn.blocks:
            out = []
            changed = False
            for inst in bb.instructions:
                si = inst.sync_info
                if si is not None and si.on_wait and len(si.on_wait) > max_waits:
                    waits = list(si.on_wait)
                    excess, keep = waits[:-max_waits], waits[-max_waits:]
                    for i in range(0, len(excess), max_waits):
                        nop = mybir.InstNoOp(
                            name=nc.get_next_instruction_name(),
                            engine=inst.engine,
                            ins=[],
                            outs=[],
                            sync_info=mybir.SyncInfo(
                                on_wait=excess[i : i + max_waits], on_update=[]
                            ),
                        )
                        nc.register_instruction(nop)
                        out.append(nop)
                    inst.sync_info = mybir.SyncInfo(
                        on_wait=keep, on_update=list(si.on_update or [])
                    )
                    changed = True
                out.append(inst)
            if changed:
                bb.instructions = out


def build_abnet(Bc: int, W: int = 512, loop_n: int = 1, skip_epilogue: bool = False, pack_l3: bool = False):
    """Build the per-core Bass graph.  Bc = per-core batch, W = batch chunk
    (free-dim width of the big matmuls, <=512 for f32 PSUM)."""
    assert Bc % 128 == 0 and Bc % W == 0
    NB = Bc // W       # batch chunks
    NC_COL = Bc // 128  # batch-major columns
    NW = W // 128      # batch-major columns per chunk
    _patch_tile_drain()

    nc = bass.Bass("TRN2")
    # ---- DRAM parameters (host-prepped layouts) ----
    d_xt = nc.dram_tensor("xt", [128, Bc], BF16, kind="ExternalInput")
    d_xbm = nc.dram_tensor("xbm", [128, NC_COL * F_IN], F32, kind="ExternalInput")
    d_w1 = nc.dram_tensor("w1", [128, H * H1], BF16, kind="ExternalInput")
    d_b1 = nc.dram_tensor("b1", [128, H * KT], F32, kind="ExternalInput")
    d_w21 = nc.dram_tensor("w21", [H, 128, KT * H1], BF16, kind="ExternalInput")
    d_w22 = nc.dram_tensor("w22", [H, 128, KT * H1], BF16, kind="ExternalInput")
    d_b21 = nc.dram_tensor("b21", [128, H * KT], F32, kind="ExternalInput")
    d_b22 = nc.dram_tensor("b22", [128, H * KT], F32, kind="ExternalInput")
    d_w31 = nc.dram_tensor("w31", [128, H * KT * C], BF16, kind="ExternalInput")
    d_w32 = nc.dram_tensor("w32", [128, H * KT * C], BF16, kind="ExternalInput")
    d_b31r = nc.dram_tensor("b31r", [128, H * C], F32, kind="ExternalInput")
    d_b32r = nc.dram_tensor("b32r", [128, H * C], F32, kind="ExternalInput")
    d_wsm = nc.dram_tensor("wsm", [128, H], F32, kind="ExternalInput")
    d_out = nc.dram_tensor("out", [128, NC_COL * C], F32, kind="ExternalOutput")

    from contextlib import ExitStack

    with tile.TileContext(nc) as tc, ExitStack() as ctx:
        const = ctx.enter_context(tc.tile_pool(name="const", bufs=1))
        w2_pool = ctx.enter_context(tc.tile_pool(name="w2", bufs=2))
        h1_pool = ctx.enter_context(tc.tile_pool(name="h1", bufs=2))
        x2_pool = ctx.enter_context(tc.tile_pool(name="x2", bufs=4))
        stage_pool = ctx.enter_context(tc.tile_pool(name="stage", bufs=1))
        ps1 = ctx.enter_context(tc.tile_pool(name="ps1", bufs=4, space="PSUM"))
        ps2 = ctx.enter_context(tc.tile_pool(name="ps2", bufs=2, space="PSUM"))
        ps3 = ctx.enter_context(tc.tile_pool(name="ps3", bufs=1, space="PSUM"))
        ep_pool = ctx.enter_context(tc.tile_pool(name="ep", bufs=1))
        epb_pool = ctx.enter_context(tc.tile_pool(name="epb", bufs=1))
        tmp_pool = ctx.enter_context(tc.tile_pool(name="tmp", bufs=12))
        bnc_pool = ctx.enter_context(tc.tile_pool(name="bnc", bufs=4))

        # ---- constant / small loads ----
        xt = const.tile([128, Bc], BF16, tag="xt")
        nc.sync.dma_start(xt[:], d_xt[:])
        xbm = const.tile([128, NC_COL * F_IN], F32, tag="xbm")
        nc.sync.dma_start(xbm[:], d_xbm[:])
        w1 = const.tile([128, H * H1], BF16, tag="w1")
        nc.sync.dma_start(w1[:], d_w1[:])
        b1 = const.tile([128, H * KT], F32, tag="b1")
        nc.sync.dma_start(b1[:], d_b1[:])
        b21 = const.tile([128, H * KT], F32, tag="b21")
        nc.sync.dma_start(b21[:], d_b21[:])
        b22 = const.tile([128, H * KT], F32, tag="b22")
        nc.sync.dma_start(b22[:], d_b22[:])
        w31 = const.tile([128, H * KT * C], BF16, tag="w31")
        nc.sync.dma_start(w31[:], d_w31[:])
        w32 = const.tile([128, H * KT * C], BF16, tag="w32")
        nc.sync.dma_start(w32[:], d_w32[:])
        b31r = const.tile([128, H * C], F32, tag="b31r")
        nc.sync.dma_start(b31r[:], d_b31r[:])
        b32r = const.tile([128, H * C], F32, tag="b32r")
        nc.sync.dma_start(b32r[:], d_b32r[:])
        wsm = const.tile([128, H], F32, tag="wsm")
        nc.sync.dma_start(wsm[:], d_wsm[:])
        ident = const.tile([128, 128], F32, tag="ident")
        make_identity(nc, ident)

        def body(_iv=None):
            # staging for per-head QP inputs, feature-major: rows 4h..4h+3 =
            # [z1, z2, s32_c0, s32_c1] of head h (z = -(x21@W31 + b31))
            S = stage_pool.tile([4 * H, Bc], F32, tag="S")

            # ---- main loop: heads x batch chunks, with layer-1 software-
            # pipelined one chunk ahead so its PSUM evictions hide under the
            # previous chunk's layer-2 matmul stream
            def emit_l1(h, bc):
                bsl = bass.ds(bc * W, W)
                h1 = h1_pool.tile([128, KT * W], BF16, tag="h1", name="h1")
                for t in range(KT):
                    # operands at partition base 32*(t%4): bass auto-derives
                    # tile_position so 4 K=4 matmuls pack onto independent
                    # 32-row groups of the PE array and stream concurrently
                    base = 32 * (t % 4)
                    p1t = ps1.tile([128, W], F32, tag="ps1", name="p1t")
                    nc.tensor.matmul(
                        p1t[:],
                        w1[base : base + F_IN, h * H1 + t * 128 : h * H1 + (t + 1) * 128],
                        xt[base : base + F_IN, bsl],
                        start=True,
                        stop=True,
                        tile_position=(base, 0),
                    )
                    # relu+bias eviction: alternate DVE / ACT
                    if t % 2 == 0:
                        nc.vector.tensor_scalar(
                            h1[:, t * W : (t + 1) * W],
                            p1t[:],
                            b1[:, h * KT + t : h * KT + t + 1],
                            0.0,
                            OP.add,
                            OP.max,
                        )
                    else:
                        nc.scalar.activation(
                            h1[:, t * W : (t + 1) * W],
                            p1t[:],
                            AF.Relu,
                            bias=b1[:, h * KT + t : h * KT + t + 1],
                        )
                return h1

            h1_tiles = {}
            for h in range(H):
                w21 = w2_pool.tile([128, KT * H1], BF16, tag="w21", name="w21")
                w22 = w2_pool.tile([128, KT * H1], BF16, tag="w22", name="w22")
                for k in range(KT):
                    nc.sync.dma_start(
                        w21[:, k * H1 : (k + 1) * H1], d_w21[h, :, k * H1 : (k + 1) * H1]
                    )
                    nc.sync.dma_start(
                        w22[:, k * H1 : (k + 1) * H1], d_w22[h, :, k * H1 : (k + 1) * H1]
                    )
                for bc in range(NB):
                    if (h, bc) not in h1_tiles:
                        h1_tiles[(h, bc)] = emit_l1(h, bc)
                    # prefetch next chunk's layer 1 into the PE stream now
                    nh, nbc = (h, bc + 1) if bc + 1 < NB else (h + 1, 0)
                    if nh < H:
                        h1_tiles[(nh, nbc)] = emit_l1(nh, nbc)
                    h1 = h1_tiles.pop((h, bc))
                    bsl = bass.ds(bc * W, W)
                    # -- layers 2+3 for each branch
                    for m, (w2, b2, w3, srow) in enumerate(
                        (
                            (w21, b21, w31, 4 * h),
                            (w22, b22, w32, 4 * h + 2),
                        )
                    ):
                        x2 = x2_pool.tile([128, KT * W], BF16, tag="x2", name="x2")
                        for t in range(KT):
                            p2t = ps2.tile([128, W], F32, tag="ps2", name="p2t")
                            for k in range(KT):
                                nc.tensor.matmul(
                                    p2t[:],
                                    w2[:, k * H1 + t * 128 : k * H1 + (t + 1) * 128],
                                    h1[:, k * W : (k + 1) * W],
                                    start=(k == 0),
                                    stop=(k == KT - 1),
                                )
                            # alternate PSUM evictions between ACT and DVE
                            if t % 2 == 0:
                                nc.scalar.activation(
                                    x2[:, t * W : (t + 1) * W],
                                    p2t[:],
                                    AF.Relu,
                                    bias=b2[:, h * KT + t : h * KT + t + 1],
                                )
                            else:
                                nc.vector.tensor_scalar(
                                    x2[:, t * W : (t + 1) * W],
                                    p2t[:],
                                    b2[:, h * KT + t : h * KT + t + 1],
                                    0.0,
                                    OP.add,
                                    OP.max,
                                )
                        p3 = ps3.tile([128, W], F32, tag="ps3", name="p3")
                        if pack_l3:
                            # 4 column-group tiles at psum bases 0/32/64/96:
                            # 4 concurrent K=128,M=2 matmuls, 2 k-tiles each
                            for t in range(KT):
                                j = t % 4
                                nc.tensor.matmul(
                                    p3[32 * j : 32 * j + C, :],
                                    w3[:, (h * KT + t) * C : (h * KT + t + 1) * C],
                                    x2[:, t * W : (t + 1) * W],
                                    start=(t < 4),
                                    stop=(t >= KT - 4),
                                    tile_position=(0, 32 * j),
                                )
                            # evict the 4 partials side by side, then sum
                            bnc = bnc_pool.tile([C, 4 * W], F32, tag="bnc", name="bnc")
                            for j in range(4):
                                nc.scalar.copy(
                                    bnc[:, j * W : (j + 1) * W],
                                    p3[32 * j : 32 * j + C, :],
                                )
                            nc.vector.tensor_add(
                                bnc[:, 0:W], bnc[:, 0:W], bnc[:, W : 2 * W]
                            )
                            nc.vector.tensor_add(
                                bnc[:, 2 * W : 3 * W], bnc[:, 2 * W : 3 * W],
                                bnc[:, 3 * W : 4 * W],
                            )
                            nc.vector.tensor_add(
                                bnc[:, 0:W], bnc[:, 0:W], bnc[:, 2 * W : 3 * W]
                            )
                            nc.sync.dma_start(S[srow : srow + C, bsl], bnc[:, 0:W])
                        else:
                            for t in range(KT):
                                nc.tensor.matmul(
                                    p3[:C, :],
                                    w3[:, (h * KT + t) * C : (h * KT + t + 1) * C],
                                    x2[:, t * W : (t + 1) * W],
                                    start=(t == 0),
                                    stop=(t == KT - 1),
                                )
                            # stage raw accumulators (bias in the epilogue);
                            # engines cannot write partition offsets that are
                            # not multiples of 32: bounce + DMA into S
                            bnc = bnc_pool.tile([C, W], F32, tag="bnc", name="bnc")
                            nc.scalar.copy(bnc[:], p3[:C, :])
                            nc.sync.dma_start(S[srow : srow + C, bsl], bnc[:])

            if skip_epilogue:
                nc.sync.dma_start(d_out[:], S[:32, : NC_COL * C])
                return
            # ---- transpose staging to batch-major: ST[p, c*40+r] = S[r, c*128+p]
            R = 4 * H
            ST = stage_pool.tile([128, NC_COL * R], F32, tag="ST")
            for c in range(NC_COL):
                pt = ps2.tile([128, R], F32, tag="pst", bufs=1)
                nc.tensor.transpose(
                    pt[:], S[:, c * 128 : (c + 1) * 128], ident[:R, :R]
                )
                nc.vector.tensor_copy(ST[:, c * R : (c + 1) * R], pt[:])

            # q(j) = [128, H, NC_COL] strided view of quantity j for all heads
            STr = ST.rearrange("p (c g j) -> p j g c", g=H, j=4)
            xbm3 = xbm.rearrange("p (c f) -> p f c", f=F_IN)
            NCH = H * NC_COL

            def q(j):
                return STr[:, j]

            def ep(tag, pool=ep_pool):
                return pool.tile([128, NC_COL], F32, tag=tag, name=tag)

            def tmp():
                return tmp_pool.tile([128, NC_COL], F32, tag="tmp", name="tmp")

            def big(tag):
                tl = epb_pool.tile([128, NCH], F32, tag=tag, name=tag)
                return tl, tl.rearrange("p (g c) -> p g c", g=H)

            def bc3(t):
                # [128, NC_COL] -> [128, H, NC_COL] head-broadcast
                return t.unsqueeze(1).broadcast_to((128, H, NC_COL))

            # ---- geometry (batch-major, denormalized positions from host) ----
            px, py, th, v = (xbm3[:, f, :] for f in range(4))
            st_, ct_, dx, dy = ep("st"), ep("ct"), ep("dx"), ep("dy")
            PI = float(np.pi)

            def wrap_to_pi(dst, src, folds=3):
                # dst = src - 2*pi*k in [-pi, pi]; handles |src| <= (2*folds+1)*pi
                c = tmp()
                nc.vector.tensor_scalar(c[:], src, PI, None, OP.is_gt)
                nc.vector.scalar_tensor_tensor(dst[:], c[:], -2 * PI, src, OP.mult, OP.add)
                for _ in range(folds):
                    c = tmp()
                    nc.vector.tensor_scalar(c[:], dst[:], -PI, None, OP.is_lt)
                    nc.vector.scalar_tensor_tensor(dst[:], c[:], 2 * PI, dst[:], OP.mult, OP.add)
                    c = tmp()
                    nc.vector.tensor_scalar(c[:], dst[:], PI, None, OP.is_gt)
                    nc.vector.scalar_tensor_tensor(dst[:], c[:], -2 * PI, dst[:], OP.mult, OP.add)

            thr = ep("thr")
            wrap_to_pi(thr, th)
            nc.scalar.activation(st_[:], thr[:], AF.Sin)
            nc.vector.tensor_scalar_add(thr[:], th, PI / 2)
            wrap_to_pi(thr, thr[:])
            nc.scalar.activation(ct_[:], thr[:], AF.Sin)
            nc.vector.tensor_scalar_add(dx[:], px, -OBS_X)
            nc.vector.tensor_scalar_add(dy[:], py, -OBS_Y)
            vst2, vct2 = ep("vst2"), ep("vct2")
            t0 = tmp()
            nc.vector.tensor_mul(t0[:], v, st_[:])
            nc.vector.tensor_scalar_mul(vst2[:], t0[:], 2.0)
            t0 = tmp()
            nc.vector.tensor_mul(t0[:], v, ct_[:])
            nc.vector.tensor_scalar_mul(vct2[:], t0[:], 2.0)
            barrier, bdot, lf2b = ep("barrier"), ep("bdot"), ep("lf2b")
            ta, tb = tmp(), tmp()
            nc.vector.tensor_mul(ta[:], dx[:], dx[:])
            nc.vector.tensor_mul(tb[:], dy[:], dy[:])
            nc.vector.scalar_tensor_tensor(
                barrier[:], ta[:], -(RAD * RAD), tb[:], OP.add, OP.add
            )
            ta, tb = tmp(), tmp()
            nc.vector.tensor_mul(ta[:], dx[:], vct2[:])
            nc.vector.tensor_mul(tb[:], dy[:], vst2[:])
            nc.vector.tensor_add(bdot[:], ta[:], tb[:])
            ta = tmp()
            nc.vector.tensor_mul(ta[:], v, v)
            nc.vector.tensor_scalar_mul(lf2b[:], ta[:], 2.0)
            G1, G2, invgg = ep("G1"), ep("G2"), ep("invgg")
            ta, tb = tmp(), tmp()
            nc.vector.tensor_mul(ta[:], dx[:], vst2[:])
            nc.vector.tensor_mul(tb[:], dy[:], vct2[:])
            nc.vector.tensor_sub(G1[:], ta[:], tb[:])
            ta, tb = tmp(), tmp()
            nc.vector.tensor_mul(ta[:], dx[:], ct_[:])
            nc.vector.tensor_mul(tb[:], dy[:], st_[:])
            nc.vector.tensor_add(ta[:], ta[:], tb[:])
            nc.vector.tensor_scalar_mul(G2[:], ta[:], -2.0)
            ta, tb = tmp(), tmp()
            nc.vector.tensor_mul(ta[:], G1[:], G1[:])
            nc.vector.tensor_mul(tb[:], G2[:], G2[:])
            nc.vector.scalar_tensor_tensor(ta[:], ta[:], 1e-12, tb[:], OP.add, OP.add)
            nc.vector.reciprocal(invgg[:], ta[:])

            # ---- QP for all heads at once on [128, H*NC_COL] tiles ----
            b31c0 = b31r[:, 0 : 2 * H : 2].to_broadcast((128, H, NC_COL))
            b31c1 = b31r[:, 1 : 2 * H : 2].to_broadcast((128, H, NC_COL))
            b32c0 = b32r[:, 0 : 2 * H : 2].to_broadcast((128, H, NC_COL))
            w_all = wsm[:, 0:H].to_broadcast((128, H, NC_COL))

            # z = -(s31 + b31)
            z1t, z1a = big("z1a")
            nc.vector.scalar_tensor_tensor(z1a, q(0), -1.0, b31c0, OP.mult, OP.subtract)
            z2t, z2a = big("z2a")
            nc.vector.scalar_tensor_tensor(z2a, q(1), -1.0, b31c1, OP.mult, OP.subtract)
            # a = 4*sigmoid(s32_c0 + b32_c0) for all heads
            at, aa = big("aa")
            nc.vector.tensor_tensor(aa, q(2), b32c0, OP.add)
            nc.scalar.activation(at[:], at[:], AF.Sigmoid)
            nc.vector.tensor_scalar_mul(at[:], at[:], 4.0)
            # head 0: p1 = a[g=0]; then a[g=0] is replaced by col-1 sigmoid
            p1v = ep("p1v")
            nc.vector.tensor_copy(p1v[:], at[:, 0:NC_COL])
            h0a = tmp()
            nc.vector.tensor_scalar(h0a[:], STr[:, 3, 0, :], b32r[:, 1:2], None, OP.add)
            nc.scalar.activation(h0a[:], h0a[:], AF.Sigmoid)
            nc.vector.tensor_scalar_mul(at[:, 0:NC_COL], h0a[:], 4.0)
            p1b = bc3(p1v)

            # h_qp = lf2b + (p1+a)*bdot + p1*a*barrier
            smt, sm3 = big("smt")
            mut, mu3 = big("mut")
            nc.vector.tensor_tensor(sm3, p1b, aa, OP.add)
            nc.vector.tensor_tensor(mu3, p1b, aa, OP.mult)
            nc.vector.tensor_tensor(sm3, sm3, bc3(bdot), OP.mult)
            nc.vector.tensor_tensor(mu3, mu3, bc3(barrier), OP.mult)
            nc.vector.tensor_add(smt[:], smt[:], mut[:])
            nc.vector.tensor_tensor(sm3, sm3, bc3(lf2b), OP.add)
            # gz = G1*z1 + G2*z2 ; lam = relu(gz - hqp) * invgg
            gat, ga3 = big("gat")
            nc.vector.tensor_tensor(ga3, bc3(G1), z1a, OP.mult)
            nc.vector.tensor_tensor(mu3, bc3(G2), z2a, OP.mult)
            nc.vector.tensor_add(gat[:], gat[:], mut[:])
            nc.vector.tensor_sub(gat[:], gat[:], smt[:])
            nc.vector.tensor_relu(gat[:], gat[:])
            nc.vector.tensor_tensor(ga3, ga3, bc3(invgg), OP.mult)  # lam
            # weighted sums over heads (w_all applied, then pairwise tree)
            nc.vector.tensor_tensor(z1a, z1a, w_all, OP.mult)
            nc.vector.tensor_tensor(z2a, z2a, w_all, OP.mult)
            nc.vector.tensor_tensor(ga3, ga3, w_all, OP.mult)
            NC2 = NC_COL

            def tree_sum(t):
                nc.vector.tensor_add(t[:, 0 : 5 * NC2], t[:, 0 : 5 * NC2], t[:, 5 * NC2 : 10 * NC2])
                nc.vector.tensor_add(t[:, 0:NC2], t[:, 0:NC2], t[:, 4 * NC2 : 5 * NC2])
                nc.vector.tensor_add(t[:, 0 : 2 * NC2], t[:, 0 : 2 * NC2], t[:, 2 * NC2 : 4 * NC2])
                nc.vector.tensor_add(t[:, 0:NC2], t[:, 0:NC2], t[:, NC2 : 2 * NC2])

            tree_sum(z1t)
            tree_sum(z2t)
            tree_sum(gat)

            # ---- u = acc_z - acc_lam * G ; write interleaved [128, (c,2)] ----
            U = stage_pool.tile([128, NC_COL * C], F32, tag="U")
            U3 = U.rearrange("p (c ch) -> p ch c", ch=C)
            ta = tmp()
            nc.vector.tensor_mul(ta[:], gat[:, 0:NC_COL], G1[:])
            nc.vector.tensor_sub(U3[:, 0, :], z1t[:, 0:NC_COL], ta[:])
            tb = tmp()
            nc.vector.tensor_mul(tb[:], gat[:, 0:NC_COL], G2[:])
            nc.vector.tensor_sub(U3[:, 1, :], z2t[:, 0:NC_COL], tb[:])
            nc.sync.dma_start(d_out[:], U[:])

        if loop_n == 1:
            body()
        else:
            with tc.For_i(0, loop_n, 1) as _iv:
                body(_iv)


    _split_excess_waits(nc)
    return nc


# ---------------- host-side preparation ----------------


def _prep_shared(W1, b1, W21, b21, W22, b22, W31, b31, W32, b32, wt):
    bf = ml_dtypes.bfloat16
    f32 = np.float32
    p = {}
    w1flat = np.asarray(W1, f32).transpose(1, 0, 2).reshape(F_IN, H * H1)
    w14 = np.zeros((128, H * H1), f32)
    for g in range(4):
        w14[32 * g : 32 * g + F_IN] = w1flat
    p["w1"] = np.ascontiguousarray(w14).astype(bf)
    for nm, b in (("b1", b1), ("b21", b21), ("b22", b22)):
        p[nm] = np.ascontiguousarray(
            np.asarray(b, f32).reshape(H, KT, 128).transpose(2, 0, 1).reshape(128, H * KT)
        )
    for nm, w in (("w21", W21), ("w22", W22)):
        p[nm] = np.ascontiguousarray(
            np.asarray(w, f32)
            .reshape(H, KT, 128, H1)
            .transpose(0, 2, 1, 3)
            .reshape(H, 128, KT * H1)
        ).astype(bf)
    for nm, w in (("w31", W31), ("w32", W32)):
        p[nm] = np.ascontiguousarray(
            np.asarray(w, f32)
            .reshape(H, KT, 128, C)
            .transpose(2, 0, 1, 3)
            .reshape(128, H * KT * C)
        ).astype(bf)
    p["b31r"] = np.ascontiguousarray(
        np.broadcast_to(np.asarray(b31, f32).reshape(1, H * C), (128, H * C))
    )
    p["b32r"] = np.ascontiguousarray(
        np.broadcast_to(np.asarray(b32, f32).reshape(1, H * C), (128, H * C))
    )
    w = np.asarray(wt, np.float64)
    e = np.exp(w - w.max())
    p["wsm"] = np.ascontiguousarray(
        np.broadcast_to((e / e.sum()).astype(f32), (128, H))
    )
    return p


def _prep_core(x_shard, xd_shard):
    bf = ml_dtypes.bfloat16
    Bc = x_shard.shape[0]
    Bc_ = x_shard.shape[0]
    xt4 = np.zeros((128, Bc_), np.float32)
    for g in range(4):
        xt4[32 * g : 32 * g + F_IN] = x_shard.T
    return {
        "xt": np.ascontiguousarray(xt4).astype(bf),
        "xbm": np.ascontiguousarray(
            xd_shard.reshape(Bc // 128, 128, F_IN).transpose(1, 0, 2).reshape(128, -1)
        ),
    }


def _gather_out(U, Bc):
    return np.ascontiguousarray(
        U.reshape(128, Bc // 128, C).transpose(1, 0, 2).reshape(Bc, C)
    )


def kernel(x, W1, b1, W21, b21, W22, b22, W31, b31, W32, b32, wt, mean, std,
           sgn=None, itr=None, **_unused):
    x = np.asarray(x, np.float32)
    B = x.shape[0]
    assert B % N_CORES == 0
    Bc = B // N_CORES
    xd = x * np.asarray(std, np.float32) + np.asarray(mean, np.float32)

    shared = _prep_shared(W1, b1, W21, b21, W22, b22, W31, b31, W32, b32, wt)
    in_maps = []
    for i in range(N_CORES):
        rows = slice(i * Bc, (i + 1) * Bc)
        m = dict(shared)
        m.update(_prep_core(x[rows], xd[rows]))
        in_maps.append(m)

    nc = build_abnet(Bc)
    res = run_bass_kernel_spmd(nc, in_maps, core_ids=list(range(N_CORES)))
    out = np.concatenate(
        [_gather_out(np.asarray(res.results[i]["out"], np.float32), Bc)
         for i in range(N_CORES)],
        axis=0,
    )
    return out



# revision 11
# speedup vs baseline: 1.6959x; 1.6959x over previous
"""Trainium2 Bass kernel for nn_ABNet: 10-head MLP ensemble + per-sample QP.

Reference computation (per sample, all heads):
  h1  = relu(x @ W1[h] + b1[h])            x:[B,4]  -> [B,1024]
  x21 = relu(h1 @ W21[h] + b21[h])         -> [B,1024]
  x22 = relu(h1 @ W22[h] + b22[h])         -> [B,1024]
  x31 = x21 @ W31[h] + b31[h]              -> [B,2]
  x32 = 4*sigmoid(x22 @ W32[h] + b32[h])   -> [B,2]
  + closed-form single-constraint QP epilogue, softmax(wt) ensemble.

Strategy: pure data parallel over batch across 8 NeuronCores (B=32768 ->
4096/core).  Feature-major layout on chip (hidden dim on partitions, batch
on the free axis) so all matmuls use natural weight layouts with no
transposes.  The dominant layer-2 matmuls (h1 @ W21 / h1 @ W22, 88% of PE
cycles) run in fp8 e4m3 with MatmulPerfMode.DoubleRow (2 k-tiles per
instruction, 2x PE throughput); h1 is evicted straight to fp8 and W2x is
host-quantized with a x32 scale folded into the bias / epilogue constants.
The QP epilogue runs batch-major after a PE transpose of the tiny per-head
outputs.  Everything is built with the Tile framework (auto scheduling).
"""

import numpy as np
import ml_dtypes

import concourse.bass as bass
import concourse.mybir as mybir
import concourse.tile as tile
from concourse.vector_clock import ScopedClock
from concourse.masks import make_identity
from concourse.bass_utils import run_bass_kernel_spmd

BF16 = mybir.dt.bfloat16
F32 = mybir.dt.float32
FP8 = mybir.dt.float8e4
AF = mybir.ActivationFunctionType
OP = mybir.AluOpType
DR = mybir.MatmulPerfMode.DoubleRow

H, F_IN, H1, C = 10, 4, 1024, 2
KT = H1 // 128  # 8 k-tiles of the hidden dim
OBS_X, OBS_Y, RAD = 40.0, 15.0, 6.0
N_CORES = 8
B_FULL = 32768
SW = 32.0  # fp8 weight scale for W21/W22 (folded into b2x and the epilogue)

_drain_patched = False


def _patch_tile_drain():
    """This container's walrus rejects >2 sync waits on one CTRL op; move the
    Tile kernel-tail drain waits onto individual SP NOPs."""
    global _drain_patched
    if _drain_patched:
        return
    _drain_patched = True

    def _drain_and_barrier(self, tick_clock, wait_clock):
        nc = self.nc
        carrier = nc.sync.nop()
        wait_clock.add_sem_waits(
            carrier.ins, ScopedClock({None: tick_clock.global_clock})
        )
        si = carrier.ins.sync_info
        waits = list(si.on_wait) if si and si.on_wait else []
        if len(waits) > 1:
            carrier.ins.sync_info = mybir.SyncInfo(on_wait=[waits[0]], on_update=[])
            for w in waits[1:]:
                nop = nc.sync.nop()
                nop.ins.sync_info = mybir.SyncInfo(on_wait=[w], on_update=[])
        nc.sync.drain()
        nc.all_engine_barrier()
        assert self.sems is not None
        popped = nc._tile_sem_poison_stack.pop()
        assert popped is self._sem_poison
        nc.clear_and_free_semaphores(list(self.sems.allocated().values()))
        nc.all_engine_barrier()

    tile.TileContext._drain_and_barrier = _drain_and_barrier


def _split_excess_waits(nc, max_waits=1):
    """This walrus build rejects instructions carrying more than a couple of
    semaphore waits; hoist the excess onto same-engine NoOps just before."""
    for fn in nc.m.functions:
        for bb in fn.blocks:
            out = []
            changed = False
            for inst in bb.instructions:
                si = inst.sync_info
                if si is not None and si.on_wait and len(si.on_wait) > max_waits:
                    waits = list(si.on_wait)
                    excess, keep = waits[:-max_waits], waits[-max_waits:]
                    for i in range(0, len(excess), max_waits):
                        nop = mybir.InstNoOp(
                            name=nc.get_next_instruction_name(),
                            engine=inst.engine,
                            ins=[],
                            outs=[],
                            sync_info=mybir.SyncInfo(
                                on_wait=excess[i : i + max_waits], on_update=[]
                            ),
                        )
                        nc.register_instruction(nop)
                        out.append(nop)
                    inst.sync_info = mybir.SyncInfo(
                        on_wait=keep, on_update=list(si.on_update or [])
                    )
                    changed = True
                out.append(inst)
            if changed:
                bb.instructions = out


def build_abnet(Bc: int, W: int = 512, loop_n: int = 1, skip_epilogue: bool = False, pack_l3: bool = False):
    """Build the per-core Bass graph.  Bc = per-core batch, W = batch chunk
    (free-dim width of the big matmuls, <=512 for f32 PSUM)."""
    assert Bc % 128 == 0 and Bc % W == 0
    NB = Bc // W       # batch chunks
    NC_COL = Bc // 128  # batch-major columns
    NW = W // 128      # batch-major columns per chunk
    _patch_tile_drain()

    nc = bass.Bass("TRN2")
    # ---- DRAM parameters (host-prepped layouts) ----
    d_xt = nc.dram_tensor("xt", [128, Bc], BF16, kind="ExternalInput")
    d_xbm = nc.dram_tensor("xbm", [128, NC_COL * F_IN], F32, kind="ExternalInput")
    d_w1 = nc.dram_tensor("w1", [128, H * H1], BF16, kind="ExternalInput")
    d_b1 = nc.dram_tensor("b1", [128, H * KT], F32, kind="ExternalInput")
    d_w21 = nc.dram_tensor("w21", [H, 128, KT * H1], FP8, kind="ExternalInput")
    d_w22 = nc.dram_tensor("w22", [H, 128, KT * H1], FP8, kind="ExternalInput")
    d_b21 = nc.dram_tensor("b21", [128, H * KT], F32, kind="ExternalInput")
    d_b22 = nc.dram_tensor("b22", [128, H * KT], F32, kind="ExternalInput")
    d_w31 = nc.dram_tensor("w31", [128, H * KT * C], BF16, kind="ExternalInput")
    d_w32 = nc.dram_tensor("w32", [128, H * KT * C], BF16, kind="ExternalInput")
    d_b31r = nc.dram_tensor("b31r", [128, H * C], F32, kind="ExternalInput")
    d_b32r = nc.dram_tensor("b32r", [128, H * C], F32, kind="ExternalInput")
    d_wsm = nc.dram_tensor("wsm", [128, H], F32, kind="ExternalInput")
    d_out = nc.dram_tensor("out", [128, NC_COL * C], F32, kind="ExternalOutput")

    from contextlib import ExitStack

    with tile.TileContext(nc) as tc, ExitStack() as ctx:
        const = ctx.enter_context(tc.tile_pool(name="const", bufs=1))
        w2_pool = ctx.enter_context(tc.tile_pool(name="w2", bufs=2))
        h1_pool = ctx.enter_context(tc.tile_pool(name="h1", bufs=2))
        x2_pool = ctx.enter_context(tc.tile_pool(name="x2", bufs=4))
        stage_pool = ctx.enter_context(tc.tile_pool(name="stage", bufs=1))
        ps1 = ctx.enter_context(tc.tile_pool(name="ps1", bufs=4, space="PSUM"))
        ps2 = ctx.enter_context(tc.tile_pool(name="ps2", bufs=2, space="PSUM"))
        ps3 = ctx.enter_context(tc.tile_pool(name="ps3", bufs=1, space="PSUM"))
        ep_pool = ctx.enter_context(tc.tile_pool(name="ep", bufs=1))
        epb_pool = ctx.enter_context(tc.tile_pool(name="epb", bufs=1))
        tmp_pool = ctx.enter_context(tc.tile_pool(name="tmp", bufs=12))
        bnc_pool = ctx.enter_context(tc.tile_pool(name="bnc", bufs=4))

        # ---- constant / small loads ----
        xt = const.tile([128, Bc], BF16, tag="xt")
        nc.sync.dma_start(xt[:], d_xt[:])
        xbm = const.tile([128, NC_COL * F_IN], F32, tag="xbm")
        nc.sync.dma_start(xbm[:], d_xbm[:])
        w1 = const.tile([128, H * H1], BF16, tag="w1")
        nc.sync.dma_start(w1[:], d_w1[:])
        b1 = const.tile([128, H * KT], F32, tag="b1")
        nc.sync.dma_start(b1[:], d_b1[:])
        b21 = const.tile([128, H * KT], F32, tag="b21")
        nc.sync.dma_start(b21[:], d_b21[:])
        b22 = const.tile([128, H * KT], F32, tag="b22")
        nc.sync.dma_start(b22[:], d_b22[:])
        w31 = const.tile([128, H * KT * C], BF16, tag="w31")
        nc.sync.dma_start(w31[:], d_w31[:])
        w32 = const.tile([128, H * KT * C], BF16, tag="w32")
        nc.sync.dma_start(w32[:], d_w32[:])
        b31r = const.tile([128, H * C], F32, tag="b31r")
        nc.sync.dma_start(b31r[:], d_b31r[:])
        b32r = const.tile([128, H * C], F32, tag="b32r")
        nc.sync.dma_start(b32r[:], d_b32r[:])
        wsm = const.tile([128, H], F32, tag="wsm")
        nc.sync.dma_start(wsm[:], d_wsm[:])
        ident = const.tile([128, 128], F32, tag="ident")
        make_identity(nc, ident)

        def body(_iv=None):
            # staging for per-head QP inputs, feature-major: rows 4h..4h+3 =
            # [z1, z2, s32_c0, s32_c1] of head h (z = -(x21@W31 + b31))
            S = stage_pool.tile([4 * H, Bc], F32, tag="S")

            # ---- main loop: heads x batch chunks, with layer-1 software-
            # pipelined one chunk ahead so its PSUM evictions hide under the
            # previous chunk's layer-2 matmul stream
            def emit_l1(h, bc):
                bsl = bass.ds(bc * W, W)
                h1 = h1_pool.tile([128, KT, W], FP8, tag="h1", name="h1")
                for t in range(KT):
                    # operands at partition base 32*(t%4): bass auto-derives
                    # tile_position so 4 K=4 matmuls pack onto independent
                    # 32-row groups of the PE array and stream concurrently
                    base = 32 * (t % 4)
                    p1t = ps1.tile([128, W], F32, tag="ps1", name="p1t")
                    nc.tensor.matmul(
                        p1t[:],
                        w1[base : base + F_IN, h * H1 + t * 128 : h * H1 + (t + 1) * 128],
                        xt[base : base + F_IN, bsl],
                        start=True,
                        stop=True,
                        tile_position=(base, 0),
                    )
                    # relu+bias eviction straight to fp8: alternate DVE / ACT
                    if t % 2 == 0:
                        nc.vector.tensor_scalar(
                            h1[:, t, :],
                            p1t[:],
                            b1[:, h * KT + t : h * KT + t + 1],
                            0.0,
                            OP.add,
                            OP.max,
                        )
                    else:
                        nc.scalar.activation(
                            h1[:, t, :],
                            p1t[:],
                            AF.Relu,
                            bias=b1[:, h * KT + t : h * KT + t + 1],
                        )
                return h1

            h1_tiles = {}
            for h in range(H):
                w21 = w2_pool.tile([128, KT, H1], FP8, tag="w21", name="w21")
                w22 = w2_pool.tile([128, KT, H1], FP8, tag="w22", name="w22")
                for k in range(KT):
                    nc.sync.dma_start(
                        w21[:, k, :], d_w21[h, :, k * H1 : (k + 1) * H1]
                    )
                    nc.sync.dma_start(
                        w22[:, k, :], d_w22[h, :, k * H1 : (k + 1) * H1]
                    )
                for bc in range(NB):
                    if (h, bc) not in h1_tiles:
                        h1_tiles[(h, bc)] = emit_l1(h, bc)
                    # prefetch next chunk's layer 1 into the PE stream now
                    nh, nbc = (h, bc + 1) if bc + 1 < NB else (h + 1, 0)
                    if nh < H:
                        h1_tiles[(nh, nbc)] = emit_l1(nh, nbc)
                    h1 = h1_tiles.pop((h, bc))
                    bsl = bass.ds(bc * W, W)
                    # -- layers 2+3 for each branch
                    for m, (w2, b2, w3, srow) in enumerate(
                        (
                            (w21, b21, w31, 4 * h),
                            (w22, b22, w32, 4 * h + 2),
                        )
                    ):
                        x2 = x2_pool.tile([128, KT, W], BF16, tag="x2", name="x2")
                        for t in range(KT):
                            p2t = ps2.tile([128, W], F32, tag="ps2", name="p2t")
                            for j in range(KT // 2):
                                # fp8 DoubleRow: 2 k-tiles per instruction
                                nc.tensor.matmul(
                                    p2t[:],
                                    w2[:, 2 * j : 2 * j + 2, t * 128 : (t + 1) * 128],
                                    h1[:, 2 * j : 2 * j + 2, :],
                                    start=(j == 0),
                                    stop=(j == KT // 2 - 1),
                                    perf_mode=DR,
                                )
                            # alternate PSUM evictions between ACT and DVE
                            if t % 2 == 0:
                                nc.scalar.activation(
                                    x2[:, t, :],
                                    p2t[:],
                                    AF.Relu,
                                    bias=b2[:, h * KT + t : h * KT + t + 1],
                                )
                            else:
                                nc.vector.tensor_scalar(
                                    x2[:, t, :],
                                    p2t[:],
                                    b2[:, h * KT + t : h * KT + t + 1],
                                    0.0,
                                    OP.add,
                                    OP.max,
                                )
                        p3 = ps3.tile([128, W], F32, tag="ps3", name="p3")
                        if pack_l3:
                            # 4 column-group tiles at psum bases 0/32/64/96:
                            # 4 concurrent K=128,M=2 matmuls, 2 k-tiles each
                            for t in range(KT):
                                j = t % 4
                                nc.tensor.matmul(
                                    p3[32 * j : 32 * j + C, :],
                                    w3[:, (h * KT + t) * C : (h * KT + t + 1) * C],
                                    x2[:, t, :],
                                    start=(t < 4),
                                    stop=(t >= KT - 4),
                                    tile_position=(0, 32 * j),
                                )
                            # evict the 4 partials side by side, then sum
                            bnc = bnc_pool.tile([C, 4 * W], F32, tag="bnc", name="bnc")
                            for j in range(4):
                                nc.scalar.copy(
                                    bnc[:, j * W : (j + 1) * W],
                                    p3[32 * j : 32 * j + C, :],
                                )
                            nc.vector.tensor_add(
                                bnc[:, 0:W], bnc[:, 0:W], bnc[:, W : 2 * W]
                            )
                            nc.vector.tensor_add(
                                bnc[:, 2 * W : 3 * W], bnc[:, 2 * W : 3 * W],
                                bnc[:, 3 * W : 4 * W],
                            )
                            nc.vector.tensor_add(
                                bnc[:, 0:W], bnc[:, 0:W], bnc[:, 2 * W : 3 * W]
                            )
                            nc.sync.dma_start(S[srow : srow + C, bsl], bnc[:, 0:W])
                        else:
                            for t in range(KT):
                                nc.tensor.matmul(
                                    p3[:C, :],
                                    w3[:, (h * KT + t) * C : (h * KT + t + 1) * C],
                                    x2[:, t, :],
                                    start=(t == 0),
                                    stop=(t == KT - 1),
                                )
                            # stage raw accumulators (bias in the epilogue);
                            # engines cannot write partition offsets that are
                            # not multiples of 32: bounce + DMA into S
                            bnc = bnc_pool.tile([C, W], F32, tag="bnc", name="bnc")
                            nc.scalar.copy(bnc[:], p3[:C, :])
                            nc.sync.dma_start(S[srow : srow + C, bsl], bnc[:])

            if skip_epilogue:
                nc.sync.dma_start(d_out[:], S[:32, : NC_COL * C])
                return
            # ---- transpose staging to batch-major: ST[p, c*40+r] = S[r, c*128+p]
            R = 4 * H
            ST = stage_pool.tile([128, NC_COL * R], F32, tag="ST")
            for c in range(NC_COL):
                pt = ps2.tile([128, R], F32, tag="pst", bufs=1)
                nc.tensor.transpose(
                    pt[:], S[:, c * 128 : (c + 1) * 128], ident[:R, :R]
                )
                nc.vector.tensor_copy(ST[:, c * R : (c + 1) * R], pt[:])

            # q(j) = [128, H, NC_COL] strided view of quantity j for all heads
            STr = ST.rearrange("p (c g j) -> p j g c", g=H, j=4)
            xbm3 = xbm.rearrange("p (c f) -> p f c", f=F_IN)
            NCH = H * NC_COL

            def q(j):
                return STr[:, j]

            def ep(tag, pool=ep_pool):
                return pool.tile([128, NC_COL], F32, tag=tag, name=tag)

            def tmp():
                return tmp_pool.tile([128, NC_COL], F32, tag="tmp", name="tmp")

            def big(tag):
                tl = epb_pool.tile([128, NCH], F32, tag=tag, name=tag)
                return tl, tl.rearrange("p (g c) -> p g c", g=H)

            def bc3(t):
                # [128, NC_COL] -> [128, H, NC_COL] head-broadcast
                return t.unsqueeze(1).broadcast_to((128, H, NC_COL))

            # ---- geometry (batch-major, denormalized positions from host) ----
            px, py, th, v = (xbm3[:, f, :] for f in range(4))
            st_, ct_, dx, dy = ep("st"), ep("ct"), ep("dx"), ep("dy")
            PI = float(np.pi)

            def wrap_to_pi(dst, src, folds=3):
                # dst = src - 2*pi*k in [-pi, pi]; handles |src| <= (2*folds+1)*pi
                c = tmp()
                nc.vector.tensor_scalar(c[:], src, PI, None, OP.is_gt)
                nc.vector.scalar_tensor_tensor(dst[:], c[:], -2 * PI, src, OP.mult, OP.add)
                for _ in range(folds):
                    c = tmp()
                    nc.vector.tensor_scalar(c[:], dst[:], -PI, None, OP.is_lt)
                    nc.vector.scalar_tensor_tensor(dst[:], c[:], 2 * PI, dst[:], OP.mult, OP.add)
                    c = tmp()
                    nc.vector.tensor_scalar(c[:], dst[:], PI, None, OP.is_gt)
                    nc.vector.scalar_tensor_tensor(dst[:], c[:], -2 * PI, dst[:], OP.mult, OP.add)

            thr = ep("thr")
            wrap_to_pi(thr, th)
            nc.scalar.activation(st_[:], thr[:], AF.Sin)
            nc.vector.tensor_scalar_add(thr[:], th, PI / 2)
            wrap_to_pi(thr, thr[:])
            nc.scalar.activation(ct_[:], thr[:], AF.Sin)
            nc.vector.tensor_scalar_add(dx[:], px, -OBS_X)
            nc.vector.tensor_scalar_add(dy[:], py, -OBS_Y)
            vst2, vct2 = ep("vst2"), ep("vct2")
            t0 = tmp()
            nc.vector.tensor_mul(t0[:], v, st_[:])
            nc.vector.tensor_scalar_mul(vst2[:], t0[:], 2.0)
            t0 = tmp()
            nc.vector.tensor_mul(t0[:], v, ct_[:])
            nc.vector.tensor_scalar_mul(vct2[:], t0[:], 2.0)
            barrier, bdot, lf2b = ep("barrier"), ep("bdot"), ep("lf2b")
            ta, tb = tmp(), tmp()
            nc.vector.tensor_mul(ta[:], dx[:], dx[:])
            nc.vector.tensor_mul(tb[:], dy[:], dy[:])
            nc.vector.scalar_tensor_tensor(
                barrier[:], ta[:], -(RAD * RAD), tb[:], OP.add, OP.add
            )
            ta, tb = tmp(), tmp()
            nc.vector.tensor_mul(ta[:], dx[:], vct2[:])
            nc.vector.tensor_mul(tb[:], dy[:], vst2[:])
            nc.vector.tensor_add(bdot[:], ta[:], tb[:])
            ta = tmp()
            nc.vector.tensor_mul(ta[:], v, v)
            nc.vector.tensor_scalar_mul(lf2b[:], ta[:], 2.0)
            G1, G2, invgg = ep("G1"), ep("G2"), ep("invgg")
            ta, tb = tmp(), tmp()
            nc.vector.tensor_mul(ta[:], dx[:], vst2[:])
            nc.vector.tensor_mul(tb[:], dy[:], vct2[:])
            nc.vector.tensor_sub(G1[:], ta[:], tb[:])
            ta, tb = tmp(), tmp()
            nc.vector.tensor_mul(ta[:], dx[:], ct_[:])
            nc.vector.tensor_mul(tb[:], dy[:], st_[:])
            nc.vector.tensor_add(ta[:], ta[:], tb[:])
            nc.vector.tensor_scalar_mul(G2[:], ta[:], -2.0)
            ta, tb = tmp(), tmp()
            nc.vector.tensor_mul(ta[:], G1[:], G1[:])
            nc.vector.tensor_mul(tb[:], G2[:], G2[:])
            nc.vector.scalar_tensor_tensor(ta[:], ta[:], 1e-12, tb[:], OP.add, OP.add)
            nc.vector.reciprocal(invgg[:], ta[:])

            # ---- QP for all heads at once on [128, H*NC_COL] tiles ----
            b31c0 = b31r[:, 0 : 2 * H : 2].to_broadcast((128, H, NC_COL))
            b31c1 = b31r[:, 1 : 2 * H : 2].to_broadcast((128, H, NC_COL))
            b32c0 = b32r[:, 0 : 2 * H : 2].to_broadcast((128, H, NC_COL))
            w_all = wsm[:, 0:H].to_broadcast((128, H, NC_COL))

            # z = -(s31/SW + b31)  (s31 carries the x32 fp8 weight scale)
            z1t, z1a = big("z1a")
            nc.vector.scalar_tensor_tensor(z1a, q(0), -1.0 / SW, b31c0, OP.mult, OP.subtract)
            z2t, z2a = big("z2a")
            nc.vector.scalar_tensor_tensor(z2a, q(1), -1.0 / SW, b31c1, OP.mult, OP.subtract)
            # a = 4*sigmoid(s32_c0/SW + b32_c0) for all heads
            at, aa = big("aa")
            nc.vector.scalar_tensor_tensor(aa, q(2), 1.0 / SW, b32c0, OP.mult, OP.add)
            nc.scalar.activation(at[:], at[:], AF.Sigmoid)
            nc.vector.tensor_scalar_mul(at[:], at[:], 4.0)
            # head 0: p1 = a[g=0]; then a[g=0] is replaced by col-1 sigmoid
            p1v = ep("p1v")
            nc.vector.tensor_copy(p1v[:], at[:, 0:NC_COL])
            h0a = tmp()
            nc.vector.tensor_scalar(h0a[:], STr[:, 3, 0, :], 1.0 / SW, b32r[:, 1:2], OP.mult, OP.add)
            nc.scalar.activation(h0a[:], h0a[:], AF.Sigmoid)
            nc.vector.tensor_scalar_mul(at[:, 0:NC_COL], h0a[:], 4.0)
            p1b = bc3(p1v)

            # h_qp = lf2b + (p1+a)*bdot + p1*a*barrier
            smt, sm3 = big("smt")
            mut, mu3 = big("mut")
            nc.vector.tensor_tensor(sm3, p1b, aa, OP.add)
            nc.vector.tensor_tensor(mu3, p1b, aa, OP.mult)
            nc.vector.tensor_tensor(sm3, sm3, bc3(bdot), OP.mult)
            nc.vector.tensor_tensor(mu3, mu3, bc3(barrier), OP.mult)
            nc.vector.tensor_add(smt[:], smt[:], mut[:])
            nc.vector.tensor_tensor(sm3, sm3, bc3(lf2b), OP.add)
            # gz = G1*z1 + G2*z2 ; lam = relu(gz - hqp) * invgg
            gat, ga3 = big("gat")
            nc.vector.tensor_tensor(ga3, bc3(G1), z1a, OP.mult)
            nc.vector.tensor_tensor(mu3, bc3(G2), z2a, OP.mult)
            nc.vector.tensor_add(gat[:], gat[:], mut[:])
            nc.vector.tensor_sub(gat[:], gat[:], smt[:])
            nc.vector.tensor_relu(gat[:], gat[:])
            nc.vector.tensor_tensor(ga3, ga3, bc3(invgg), OP.mult)  # lam
            # weighted sums over heads (w_all applied, then pairwise tree)
            nc.vector.tensor_tensor(z1a, z1a, w_all, OP.mult)
            nc.vector.tensor_tensor(z2a, z2a, w_all, OP.mult)
            nc.vector.tensor_tensor(ga3, ga3, w_all, OP.mult)
            NC2 = NC_COL

            def tree_sum(t):
                nc.vector.tensor_add(t[:, 0 : 5 * NC2], t[:, 0 : 5 * NC2], t[:, 5 * NC2 : 10 * NC2])
                nc.vector.tensor_add(t[:, 0:NC2], t[:, 0:NC2], t[:, 4 * NC2 : 5 * NC2])
                nc.vector.tensor_add(t[:, 0 : 2 * NC2], t[:, 0 : 2 * NC2], t[:, 2 * NC2 : 4 * NC2])
                nc.vector.tensor_add(t[:, 0:NC2], t[:, 0:NC2], t[:, NC2 : 2 * NC2])

            tree_sum(z1t)
            tree_sum(z2t)
            tree_sum(gat)

            # ---- u = acc_z - acc_lam * G ; write interleaved [128, (c,2)] ----
            U = stage_pool.tile([128, NC_COL * C], F32, tag="U")
            U3 = U.rearrange("p (c ch) -> p ch c", ch=C)
            ta = tmp()
            nc.vector.tensor_mul(ta[:], gat[:, 0:NC_COL], G1[:])
            nc.vector.tensor_sub(U3[:, 0, :], z1t[:, 0:NC_COL], ta[:])
            tb = tmp()
            nc.vector.tensor_mul(tb[:], gat[:, 0:NC_COL], G2[:])
            nc.vector.tensor_sub(U3[:, 1, :], z2t[:, 0:NC_COL], tb[:])
            nc.sync.dma_start(d_out[:], U[:])

        if loop_n == 1:
            body()
        else:
            with tc.For_i(0, loop_n, 1) as _iv:
                body(_iv)


    _split_excess_waits(nc)
    return nc


# ---------------- host-side preparation ----------------


def _prep_shared(W1, b1, W21, b21, W22, b22, W31, b31, W32, b32, wt):
    bf = ml_dtypes.bfloat16
    f32 = np.float32
    p = {}
    w1flat = np.asarray(W1, f32).transpose(1, 0, 2).reshape(F_IN, H * H1)
    w14 = np.zeros((128, H * H1), f32)
    for g in range(4):
        w14[32 * g : 32 * g + F_IN] = w1flat
    p["w1"] = np.ascontiguousarray(w14).astype(bf)
    fp8 = ml_dtypes.float8_e4m3
    for nm, b, s in (("b1", b1, 1.0), ("b21", b21, SW), ("b22", b22, SW)):
        p[nm] = np.ascontiguousarray(
            np.asarray(b, f32).reshape(H, KT, 128).transpose(2, 0, 1).reshape(128, H * KT)
            * f32(s)
        )
    for nm, w in (("w21", W21), ("w22", W22)):
        p[nm] = np.ascontiguousarray(
            np.clip(
                np.asarray(w, f32)
                .reshape(H, KT, 128, H1)
                .transpose(0, 2, 1, 3)
                .reshape(H, 128, KT * H1)
                * f32(SW),
                -240.0,
                240.0,
            )
        ).astype(fp8)
    for nm, w in (("w31", W31), ("w32", W32)):
        p[nm] = np.ascontiguousarray(
            np.asarray(w, f32)
            .reshape(H, KT, 128, C)
            .transpose(2, 0, 1, 3)
            .reshape(128, H * KT * C)
        ).astype(bf)
    p["b31r"] = np.ascontiguousarray(
        np.broadcast_to(np.asarray(b31, f32).reshape(1, H * C), (128, H * C))
    )
    p["b32r"] = np.ascontiguousarray(
        np.broadcast_to(np.asarray(b32, f32).reshape(1, H * C), (128, H * C))
    )
    w = np.asarray(wt, np.float64)
    e = np.exp(w - w.max())
    p["wsm"] = np.ascontiguousarray(
        np.broadcast_to((e / e.sum()).astype(f32), (128, H))
    )
    return p


def _prep_core(x_shard, xd_shard):
    bf = ml_dtypes.bfloat16
    Bc = x_shard.shape[0]
    Bc_ = x_shard.shape[0]
    xt4 = np.zeros((128, Bc_), np.float32)
    for g in range(4):
        xt4[32 * g : 32 * g + F_IN] = x_shard.T
    return {
        "xt": np.ascontiguousarray(xt4).astype(bf),
        "xbm": np.ascontiguousarray(
            xd_shard.reshape(Bc // 128, 128, F_IN).transpose(1, 0, 2).reshape(128, -1)
        ),
    }


def _gather_out(U, Bc):
    return np.ascontiguousarray(
        U.reshape(128, Bc // 128, C).transpose(1, 0, 2).reshape(Bc, C)
    )


def kernel(x, W1, b1, W21, b21, W22, b22, W31, b31, W32, b32, wt, mean, std,
           sgn=None, itr=None, **_unused):
    x = np.asarray(x, np.float32)
    B = x.shape[0]
    assert B % N_CORES == 0
    Bc = B // N_CORES
    xd = x * np.asarray(std, np.float32) + np.asarray(mean, np.float32)

    shared = _prep_shared(W1, b1, W21, b21, W22, b22, W31, b31, W32, b32, wt)
    in_maps = []
    for i in range(N_CORES):
        rows = slice(i * Bc, (i + 1) * Bc)
        m = dict(shared)
        m.update(_prep_core(x[rows], xd[rows]))
        in_maps.append(m)

    nc = build_abnet(Bc)
    res = run_bass_kernel_spmd(nc, in_maps, core_ids=list(range(N_CORES)))
    out = np.concatenate(
        [_gather_out(np.asarray(res.results[i]["out"], np.float32), Bc)
         for i in range(N_CORES)],
        axis=0,
    )
    return out



# revision 16
# speedup vs baseline: 1.7678x; 1.0424x over previous
"""Trainium2 Bass kernel for nn_ABNet: 10-head MLP ensemble + per-sample QP.

Reference computation (per sample, all heads):
  h1  = relu(x @ W1[h] + b1[h])            x:[B,4]  -> [B,1024]
  x21 = relu(h1 @ W21[h] + b21[h])         -> [B,1024]
  x22 = relu(h1 @ W22[h] + b22[h])         -> [B,1024]
  x31 = x21 @ W31[h] + b31[h]              -> [B,2]
  x32 = 4*sigmoid(x22 @ W32[h] + b32[h])   -> [B,2]
  + closed-form single-constraint QP epilogue, softmax(wt) ensemble.

Strategy: pure data parallel over batch across 8 NeuronCores (B=32768 ->
4096/core).  Feature-major layout on chip (hidden dim on partitions, batch
on the free axis) so all matmuls use natural weight layouts with no
transposes.  The dominant layer-2 matmuls (h1 @ W21 / h1 @ W22, 88% of PE
cycles) run in fp8 e4m3 with MatmulPerfMode.DoubleRow (2 k-tiles per
instruction, 2x PE throughput); h1 is evicted straight to fp8 and W2x is
host-quantized with a x32 scale folded into the bias / epilogue constants.
The QP epilogue runs batch-major after a PE transpose of the tiny per-head
outputs.  Everything is built with the Tile framework (auto scheduling).
"""

import numpy as np
import ml_dtypes

import concourse.bass as bass
import concourse.mybir as mybir
import concourse.tile as tile
from concourse.vector_clock import ScopedClock
from concourse.masks import make_identity
from concourse.bass_utils import run_bass_kernel_spmd

BF16 = mybir.dt.bfloat16
F32 = mybir.dt.float32
FP8 = mybir.dt.float8e4
AF = mybir.ActivationFunctionType
OP = mybir.AluOpType
DR = mybir.MatmulPerfMode.DoubleRow

H, F_IN, H1, C = 10, 4, 1024, 2
KT = H1 // 128  # 8 k-tiles of the hidden dim
OBS_X, OBS_Y, RAD = 40.0, 15.0, 6.0
N_CORES = 8
B_FULL = 32768
SW = 32.0  # fp8 weight scale for W21/W22 (folded into b2x and the epilogue)

_drain_patched = False


def _patch_tile_drain():
    """This container's walrus rejects >2 sync waits on one CTRL op; move the
    Tile kernel-tail drain waits onto individual SP NOPs."""
    global _drain_patched
    if _drain_patched:
        return
    _drain_patched = True

    def _drain_and_barrier(self, tick_clock, wait_clock):
        nc = self.nc
        carrier = nc.sync.nop()
        wait_clock.add_sem_waits(
            carrier.ins, ScopedClock({None: tick_clock.global_clock})
        )
        si = carrier.ins.sync_info
        waits = list(si.on_wait) if si and si.on_wait else []
        if len(waits) > 1:
            carrier.ins.sync_info = mybir.SyncInfo(on_wait=[waits[0]], on_update=[])
            for w in waits[1:]:
                nop = nc.sync.nop()
                nop.ins.sync_info = mybir.SyncInfo(on_wait=[w], on_update=[])
        nc.sync.drain()
        nc.all_engine_barrier()
        assert self.sems is not None
        popped = nc._tile_sem_poison_stack.pop()
        assert popped is self._sem_poison
        nc.clear_and_free_semaphores(list(self.sems.allocated().values()))
        nc.all_engine_barrier()

    tile.TileContext._drain_and_barrier = _drain_and_barrier


def _split_excess_waits(nc, max_waits=1):
    """This walrus build rejects instructions carrying more than a couple of
    semaphore waits; hoist the excess onto same-engine NoOps just before."""
    for fn in nc.m.functions:
        for bb in fn.blocks:
            out = []
            changed = False
            for inst in bb.instructions:
                si = inst.sync_info
                if si is not None and si.on_wait and len(si.on_wait) > max_waits:
                    waits = list(si.on_wait)
                    excess, keep = waits[:-max_waits], waits[-max_waits:]
                    for i in range(0, len(excess), max_waits):
                        nop = mybir.InstNoOp(
                            name=nc.get_next_instruction_name(),
                            engine=inst.engine,
                            ins=[],
                            outs=[],
                            sync_info=mybir.SyncInfo(
                                on_wait=excess[i : i + max_waits], on_update=[]
                            ),
                        )
                        nc.register_instruction(nop)
                        out.append(nop)
                    inst.sync_info = mybir.SyncInfo(
                        on_wait=keep, on_update=list(si.on_update or [])
                    )
                    changed = True
                out.append(inst)
            if changed:
                bb.instructions = out


def build_abnet(Bc: int, W: int = 512, loop_n: int = 1, skip_epilogue: bool = False, pack_l3: bool = False):
    """Build the per-core Bass graph.  Bc = per-core batch, W = batch chunk
    (free-dim width of the big matmuls, <=512 for f32 PSUM)."""
    assert Bc % 128 == 0 and Bc % W == 0
    NB = Bc // W       # batch chunks
    NC_COL = Bc // 128  # batch-major columns
    NW = W // 128      # batch-major columns per chunk
    _patch_tile_drain()

    nc = bass.Bass("TRN2")
    # ---- DRAM parameters (host-prepped layouts) ----
    d_xt = nc.dram_tensor("xt", [128, Bc], BF16, kind="ExternalInput")
    d_xbm = nc.dram_tensor("xbm", [128, NC_COL * F_IN], F32, kind="ExternalInput")
    d_w1 = nc.dram_tensor("w1", [128, H * H1], BF16, kind="ExternalInput")
    d_b1 = nc.dram_tensor("b1", [128, H * KT], F32, kind="ExternalInput")
    d_w21 = nc.dram_tensor("w21", [H, 128, KT * H1], FP8, kind="ExternalInput")
    d_w22 = nc.dram_tensor("w22", [H, 128, KT * H1], FP8, kind="ExternalInput")
    d_b21 = nc.dram_tensor("b21", [128, H * KT], F32, kind="ExternalInput")
    d_b22 = nc.dram_tensor("b22", [128, H * KT], F32, kind="ExternalInput")
    d_w31 = nc.dram_tensor("w31", [128, H * KT * C], BF16, kind="ExternalInput")
    d_w32 = nc.dram_tensor("w32", [128, H * KT * C], BF16, kind="ExternalInput")
    d_b31r = nc.dram_tensor("b31r", [128, H * C], F32, kind="ExternalInput")
    d_b32r = nc.dram_tensor("b32r", [128, H * C], F32, kind="ExternalInput")
    d_wsm = nc.dram_tensor("wsm", [128, H], F32, kind="ExternalInput")
    d_out = nc.dram_tensor("out", [128, NC_COL * C], F32, kind="ExternalOutput")

    from contextlib import ExitStack

    with tile.TileContext(nc) as tc, ExitStack() as ctx:
        const = ctx.enter_context(tc.tile_pool(name="const", bufs=1))
        w2_pool = ctx.enter_context(tc.tile_pool(name="w2", bufs=2))
        h1_pool = ctx.enter_context(tc.tile_pool(name="h1", bufs=2))
        x2_pool = ctx.enter_context(tc.tile_pool(name="x2", bufs=4))
        stage_pool = ctx.enter_context(tc.tile_pool(name="stage", bufs=1))
        ps1 = ctx.enter_context(tc.tile_pool(name="ps1", bufs=2, space="PSUM"))
        ps2 = ctx.enter_context(tc.tile_pool(name="ps2", bufs=4, space="PSUM"))
        ps3 = ctx.enter_context(tc.tile_pool(name="ps3", bufs=1, space="PSUM"))
        ep_pool = ctx.enter_context(tc.tile_pool(name="ep", bufs=1))
        epb_pool = ctx.enter_context(tc.tile_pool(name="epb", bufs=1))
        tmp_pool = ctx.enter_context(tc.tile_pool(name="tmp", bufs=12))
        bnc_pool = ctx.enter_context(tc.tile_pool(name="bnc", bufs=4))

        # ---- constant / small loads ----
        xt = const.tile([128, Bc], BF16, tag="xt")
        nc.sync.dma_start(xt[:], d_xt[:])
        xbm = const.tile([128, NC_COL * F_IN], F32, tag="xbm")
        nc.sync.dma_start(xbm[:], d_xbm[:])
        w1 = const.tile([128, H * H1], BF16, tag="w1")
        nc.sync.dma_start(w1[:], d_w1[:])
        b1 = const.tile([128, H * KT], F32, tag="b1")
        nc.sync.dma_start(b1[:], d_b1[:])
        b21 = const.tile([128, H * KT], F32, tag="b21")
        nc.sync.dma_start(b21[:], d_b21[:])
        b22 = const.tile([128, H * KT], F32, tag="b22")
        nc.sync.dma_start(b22[:], d_b22[:])
        w31 = const.tile([128, H * KT * C], BF16, tag="w31")
        nc.sync.dma_start(w31[:], d_w31[:])
        w32 = const.tile([128, H * KT * C], BF16, tag="w32")
        nc.sync.dma_start(w32[:], d_w32[:])
        b31r = const.tile([128, H * C], F32, tag="b31r")
        nc.sync.dma_start(b31r[:], d_b31r[:])
        b32r = const.tile([128, H * C], F32, tag="b32r")
        nc.sync.dma_start(b32r[:], d_b32r[:])
        wsm = const.tile([128, H], F32, tag="wsm")
        nc.sync.dma_start(wsm[:], d_wsm[:])
        ident = const.tile([128, 128], F32, tag="ident")
        make_identity(nc, ident)

        def body(_iv=None):
            # staging for per-head QP inputs, feature-major: rows 4h..4h+3 =
            # [z1, z2, s32_c0, s32_c1] of head h (z = -(x21@W31 + b31))
            S = stage_pool.tile([4 * H, Bc], F32, tag="S")

            # ---- main loop: heads x batch chunks.  Layer-1 matmuls for the
            # NEXT chunk are interleaved one at a time between the current
            # chunk's layer-2 out-tile groups so the in-order PE queue never
            # stalls on an L1 PSUM eviction; evictions are pinned per engine
            # (L2 -> ACT, L1 -> DVE, L3 -> Pool) to keep queues conflict-free.
            def emit_l1_tile(h, bc, h1, t):
                bsl = bass.ds(bc * W, W)
                # operands at partition base 32*(t%4): bass auto-derives
                # tile_position so K=4 matmuls pack onto independent
                # 32-row groups of the PE array
                base = 32 * (t % 4)
                p1t = ps1.tile([128, W], F32, tag="ps1", name="p1t")
                nc.tensor.matmul(
                    p1t[:],
                    w1[base : base + F_IN, h * H1 + t * 128 : h * H1 + (t + 1) * 128],
                    xt[base : base + F_IN, bsl],
                    start=True,
                    stop=True,
                    tile_position=(base, 0),
                )
                # relu+bias eviction straight to fp8 on DVE
                nc.vector.tensor_scalar(
                    h1[:, t, :],
                    p1t[:],
                    b1[:, h * KT + t : h * KT + t + 1],
                    0.0,
                    OP.add,
                    OP.max,
                )

            h1_tiles = {}
            for h in range(H):
                w21 = w2_pool.tile([128, KT, H1], FP8, tag="w21", name="w21")
                w22 = w2_pool.tile([128, KT, H1], FP8, tag="w22", name="w22")
                for k in range(KT):
                    nc.sync.dma_start(
                        w21[:, k, :], d_w21[h, :, k * H1 : (k + 1) * H1]
                    )
                    nc.sync.dma_start(
                        w22[:, k, :], d_w22[h, :, k * H1 : (k + 1) * H1]
                    )
                for bc in range(NB):
                    if (h, bc) not in h1_tiles:
                        # startup only: emit the first chunk's L1 as a burst
                        h1_tiles[(h, bc)] = h1_pool.tile(
                            [128, KT, W], FP8, tag="h1", name="h1"
                        )
                        for t in range(KT):
                            emit_l1_tile(h, bc, h1_tiles[(h, bc)], t)
                    nh, nbc = (h, bc + 1) if bc + 1 < NB else (h + 1, 0)
                    nxt = None
                    if nh < H:
                        nxt = h1_pool.tile([128, KT, W], FP8, tag="h1", name="h1")
                        h1_tiles[(nh, nbc)] = nxt
                    h1 = h1_tiles.pop((h, bc))
                    bsl = bass.ds(bc * W, W)
                    # -- layers 2+3 for each branch
                    for m, (w2, b2, w3, srow) in enumerate(
                        (
                            (w21, b21, w31, 4 * h),
                            (w22, b22, w32, 4 * h + 2),
                        )
                    ):
                        x2 = x2_pool.tile([128, KT, W], BF16, tag="x2", name="x2")
                        for t in range(KT):
                            p2t = ps2.tile([128, W], F32, tag="ps2", name="p2t")
                            for j in range(KT // 2):
                                # fp8 DoubleRow: 2 k-tiles per instruction
                                nc.tensor.matmul(
                                    p2t[:],
                                    w2[:, 2 * j : 2 * j + 2, t * 128 : (t + 1) * 128],
                                    h1[:, 2 * j : 2 * j + 2, :],
                                    start=(j == 0),
                                    stop=(j == KT // 2 - 1),
                                    perf_mode=DR,
                                )
                            # PSUM eviction pinned to ACT
                            nc.scalar.activation(
                                x2[:, t, :],
                                p2t[:],
                                AF.Relu,
                                bias=b2[:, h * KT + t : h * KT + t + 1],
                            )
                            # one next-chunk L1 matmul between out-tile groups
                            if nxt is not None:
                                li = t // 2 if m == 0 and t % 2 == 1 else (
                                    4 + t // 2 if m == 1 and t % 2 == 0 else None
                                )
                                if li is not None:
                                    emit_l1_tile(nh, nbc, nxt, li)
                        p3 = ps3.tile([128, W], F32, tag="ps3", name="p3")
                        if pack_l3:
                            # 4 column-group tiles at psum bases 0/32/64/96:
                            # 4 concurrent K=128,M=2 matmuls, 2 k-tiles each
                            for t in range(KT):
                                j = t % 4
                                nc.tensor.matmul(
                                    p3[32 * j : 32 * j + C, :],
                                    w3[:, (h * KT + t) * C : (h * KT + t + 1) * C],
                                    x2[:, t, :],
                                    start=(t < 4),
                                    stop=(t >= KT - 4),
                                    tile_position=(0, 32 * j),
                                )
                            # evict the 4 partials side by side, then sum
                            bnc = bnc_pool.tile([C, 4 * W], F32, tag="bnc", name="bnc")
                            for j in range(4):
                                nc.scalar.copy(
                                    bnc[:, j * W : (j + 1) * W],
                                    p3[32 * j : 32 * j + C, :],
                                )
                            nc.vector.tensor_add(
                                bnc[:, 0:W], bnc[:, 0:W], bnc[:, W : 2 * W]
                            )
                            nc.vector.tensor_add(
                                bnc[:, 2 * W : 3 * W], bnc[:, 2 * W : 3 * W],
                                bnc[:, 3 * W : 4 * W],
                            )
                            nc.vector.tensor_add(
                                bnc[:, 0:W], bnc[:, 0:W], bnc[:, 2 * W : 3 * W]
                            )
                            nc.sync.dma_start(S[srow : srow + C, bsl], bnc[:, 0:W])
                        else:
                            for t in range(KT):
                                nc.tensor.matmul(
                                    p3[:C, :],
                                    w3[:, (h * KT + t) * C : (h * KT + t + 1) * C],
                                    x2[:, t, :],
                                    start=(t == 0),
                                    stop=(t == KT - 1),
                                )
                            # stage raw accumulators (bias in the epilogue);
                            # engines cannot write partition offsets that are
                            # not multiples of 32: bounce + DMA into S
                            # (GPSIMD cannot read PSUM, so this rides DVE)
                            bnc = bnc_pool.tile([C, W], F32, tag="bnc", name="bnc")
                            nc.vector.tensor_copy(bnc[:], p3[:C, :])
                            nc.sync.dma_start(S[srow : srow + C, bsl], bnc[:])

            if skip_epilogue:
                nc.sync.dma_start(d_out[:], S[:32, : NC_COL * C])
                return
            # ---- transpose staging to batch-major: ST[p, c*40+r] = S[r, c*128+p]
            R = 4 * H
            ST = stage_pool.tile([128, NC_COL * R], F32, tag="ST")
            for c in range(NC_COL):
                pt = ps2.tile([128, R], F32, tag="pst", bufs=1)
                nc.tensor.transpose(
                    pt[:], S[:, c * 128 : (c + 1) * 128], ident[:R, :R]
                )
                nc.vector.tensor_copy(ST[:, c * R : (c + 1) * R], pt[:])

            # q(j) = [128, H, NC_COL] strided view of quantity j for all heads
            STr = ST.rearrange("p (c g j) -> p j g c", g=H, j=4)
            xbm3 = xbm.rearrange("p (c f) -> p f c", f=F_IN)
            NCH = H * NC_COL

            def q(j):
                return STr[:, j]

            def ep(tag, pool=ep_pool):
                return pool.tile([128, NC_COL], F32, tag=tag, name=tag)

            def tmp():
                return tmp_pool.tile([128, NC_COL], F32, tag="tmp", name="tmp")

            def big(tag):
                tl = epb_pool.tile([128, NCH], F32, tag=tag, name=tag)
                return tl, tl.rearrange("p (g c) -> p g c", g=H)

            def bc3(t):
                # [128, NC_COL] -> [128, H, NC_COL] head-broadcast
                return t.unsqueeze(1).broadcast_to((128, H, NC_COL))

            # ---- geometry (batch-major, denormalized positions from host) ----
            px, py, th, v = (xbm3[:, f, :] for f in range(4))
            st_, ct_, dx, dy = ep("st"), ep("ct"), ep("dx"), ep("dy")
            PI = float(np.pi)

            def wrap_to_pi(dst, src, folds=3):
                # dst = src - 2*pi*k in [-pi, pi]; handles |src| <= (2*folds+1)*pi
                c = tmp()
                nc.vector.tensor_scalar(c[:], src, PI, None, OP.is_gt)
                nc.vector.scalar_tensor_tensor(dst[:], c[:], -2 * PI, src, OP.mult, OP.add)
                for _ in range(folds):
                    c = tmp()
                    nc.vector.tensor_scalar(c[:], dst[:], -PI, None, OP.is_lt)
                    nc.vector.scalar_tensor_tensor(dst[:], c[:], 2 * PI, dst[:], OP.mult, OP.add)
                    c = tmp()
                    nc.vector.tensor_scalar(c[:], dst[:], PI, None, OP.is_gt)
                    nc.vector.scalar_tensor_tensor(dst[:], c[:], -2 * PI, dst[:], OP.mult, OP.add)

            thr = ep("thr")
            wrap_to_pi(thr, th)
            nc.scalar.activation(st_[:], thr[:], AF.Sin)
            nc.vector.tensor_scalar_add(thr[:], th, PI / 2)
            wrap_to_pi(thr, thr[:])
            nc.scalar.activation(ct_[:], thr[:], AF.Sin)
            nc.vector.tensor_scalar_add(dx[:], px, -OBS_X)
            nc.vector.tensor_scalar_add(dy[:], py, -OBS_Y)
            vst2, vct2 = ep("vst2"), ep("vct2")
            t0 = tmp()
            nc.vector.tensor_mul(t0[:], v, st_[:])
            nc.vector.tensor_scalar_mul(vst2[:], t0[:], 2.0)
            t0 = tmp()
            nc.vector.tensor_mul(t0[:], v, ct_[:])
            nc.vector.tensor_scalar_mul(vct2[:], t0[:], 2.0)
            barrier, bdot, lf2b = ep("barrier"), ep("bdot"), ep("lf2b")
            ta, tb = tmp(), tmp()
            nc.vector.tensor_mul(ta[:], dx[:], dx[:])
            nc.vector.tensor_mul(tb[:], dy[:], dy[:])
            nc.vector.scalar_tensor_tensor(
                barrier[:], ta[:], -(RAD * RAD), tb[:], OP.add, OP.add
            )
            ta, tb = tmp(), tmp()
            nc.vector.tensor_mul(ta[:], dx[:], vct2[:])
            nc.vector.tensor_mul(tb[:], dy[:], vst2[:])
            nc.vector.tensor_add(bdot[:], ta[:], tb[:])
            ta = tmp()
            nc.vector.tensor_mul(ta[:], v, v)
            nc.vector.tensor_scalar_mul(lf2b[:], ta[:], 2.0)
            G1, G2, invgg = ep("G1"), ep("G2"), ep("invgg")
            ta, tb = tmp(), tmp()
            nc.vector.tensor_mul(ta[:], dx[:], vst2[:])
            nc.vector.tensor_mul(tb[:], dy[:], vct2[:])
            nc.vector.tensor_sub(G1[:], ta[:], tb[:])
            ta, tb = tmp(), tmp()
            nc.vector.tensor_mul(ta[:], dx[:], ct_[:])
            nc.vector.tensor_mul(tb[:], dy[:], st_[:])
            nc.vector.tensor_add(ta[:], ta[:], tb[:])
            nc.vector.tensor_scalar_mul(G2[:], ta[:], -2.0)
            ta, tb = tmp(), tmp()
            nc.vector.tensor_mul(ta[:], G1[:], G1[:])
            nc.vector.tensor_mul(tb[:], G2[:], G2[:])
            nc.vector.scalar_tensor_tensor(ta[:], ta[:], 1e-12, tb[:], OP.add, OP.add)
            nc.vector.reciprocal(invgg[:], ta[:])

            # ---- QP for all heads at once on [128, H*NC_COL] tiles ----
            b31c0 = b31r[:, 0 : 2 * H : 2].to_broadcast((128, H, NC_COL))
            b31c1 = b31r[:, 1 : 2 * H : 2].to_broadcast((128, H, NC_COL))
            b32c0 = b32r[:, 0 : 2 * H : 2].to_broadcast((128, H, NC_COL))
            w_all = wsm[:, 0:H].to_broadcast((128, H, NC_COL))

            # z = -(s31/SW + b31)  (s31 carries the x32 fp8 weight scale)
            z1t, z1a = big("z1a")
            nc.vector.scalar_tensor_tensor(z1a, q(0), -1.0 / SW, b31c0, OP.mult, OP.subtract)
            z2t, z2a = big("z2a")
            nc.vector.scalar_tensor_tensor(z2a, q(1), -1.0 / SW, b31c1, OP.mult, OP.subtract)
            # a = 4*sigmoid(s32_c0/SW + b32_c0) for all heads
            at, aa = big("aa")
            nc.vector.scalar_tensor_tensor(aa, q(2), 1.0 / SW, b32c0, OP.mult, OP.add)
            nc.scalar.activation(at[:], at[:], AF.Sigmoid)
            nc.vector.tensor_scalar_mul(at[:], at[:], 4.0)
            # head 0: p1 = a[g=0]; then a[g=0] is replaced by col-1 sigmoid
            p1v = ep("p1v")
            nc.vector.tensor_copy(p1v[:], at[:, 0:NC_COL])
            h0a = tmp()
            nc.vector.tensor_scalar(h0a[:], STr[:, 3, 0, :], 1.0 / SW, b32r[:, 1:2], OP.mult, OP.add)
            nc.scalar.activation(h0a[:], h0a[:], AF.Sigmoid)
            nc.vector.tensor_scalar_mul(at[:, 0:NC_COL], h0a[:], 4.0)
            p1b = bc3(p1v)

            # h_qp = lf2b + (p1+a)*bdot + p1*a*barrier
            smt, sm3 = big("smt")
            mut, mu3 = big("mut")
            nc.vector.tensor_tensor(sm3, p1b, aa, OP.add)
            nc.vector.tensor_tensor(mu3, p1b, aa, OP.mult)
            nc.vector.tensor_tensor(sm3, sm3, bc3(bdot), OP.mult)
            nc.vector.tensor_tensor(mu3, mu3, bc3(barrier), OP.mult)
            nc.vector.tensor_add(smt[:], smt[:], mut[:])
            nc.vector.tensor_tensor(sm3, sm3, bc3(lf2b), OP.add)
            # gz = G1*z1 + G2*z2 ; lam = relu(gz - hqp) * invgg
            gat, ga3 = big("gat")
            nc.vector.tensor_tensor(ga3, bc3(G1), z1a, OP.mult)
            nc.vector.tensor_tensor(mu3, bc3(G2), z2a, OP.mult)
            nc.vector.tensor_add(gat[:], gat[:], mut[:])
            nc.vector.tensor_sub(gat[:], gat[:], smt[:])
            nc.vector.tensor_relu(gat[:], gat[:])
            nc.vector.tensor_tensor(ga3, ga3, bc3(invgg), OP.mult)  # lam
            # weighted sums over heads (w_all applied, then pairwise tree)
            nc.vector.tensor_tensor(z1a, z1a, w_all, OP.mult)
            nc.vector.tensor_tensor(z2a, z2a, w_all, OP.mult)
            nc.vector.tensor_tensor(ga3, ga3, w_all, OP.mult)
            NC2 = NC_COL

            def tree_sum(t):
                nc.vector.tensor_add(t[:, 0 : 5 * NC2], t[:, 0 : 5 * NC2], t[:, 5 * NC2 : 10 * NC2])
                nc.vector.tensor_add(t[:, 0:NC2], t[:, 0:NC2], t[:, 4 * NC2 : 5 * NC2])
                nc.vector.tensor_add(t[:, 0 : 2 * NC2], t[:, 0 : 2 * NC2], t[:, 2 * NC2 : 4 * NC2])
                nc.vector.tensor_add(t[:, 0:NC2], t[:, 0:NC2], t[:, NC2 : 2 * NC2])

            tree_sum(z1t)
            tree_sum(z2t)
            tree_sum(gat)

            # ---- u = acc_z - acc_lam * G ; write interleaved [128, (c,2)] ----
            U = stage_pool.tile([128, NC_COL * C], F32, tag="U")
            U3 = U.rearrange("p (c ch) -> p ch c", ch=C)
            ta = tmp()
            nc.vector.tensor_mul(ta[:], gat[:, 0:NC_COL], G1[:])
            nc.vector.tensor_sub(U3[:, 0, :], z1t[:, 0:NC_COL], ta[:])
            tb = tmp()
            nc.vector.tensor_mul(tb[:], gat[:, 0:NC_COL], G2[:])
            nc.vector.tensor_sub(U3[:, 1, :], z2t[:, 0:NC_COL], tb[:])
            nc.sync.dma_start(d_out[:], U[:])

        if loop_n == 1:
            body()
        else:
            with tc.For_i(0, loop_n, 1) as _iv:
                body(_iv)


    _split_excess_waits(nc)
    return nc


# ---------------- host-side preparation ----------------


def _prep_shared(W1, b1, W21, b21, W22, b22, W31, b31, W32, b32, wt):
    bf = ml_dtypes.bfloat16
    f32 = np.float32
    p = {}
    w1flat = np.asarray(W1, f32).transpose(1, 0, 2).reshape(F_IN, H * H1)
    w14 = np.zeros((128, H * H1), f32)
    for g in range(4):
        w14[32 * g : 32 * g + F_IN] = w1flat
    p["w1"] = np.ascontiguousarray(w14).astype(bf)
    fp8 = ml_dtypes.float8_e4m3
    for nm, b, s in (("b1", b1, 1.0), ("b21", b21, SW), ("b22", b22, SW)):
        p[nm] = np.ascontiguousarray(
            np.asarray(b, f32).reshape(H, KT, 128).transpose(2, 0, 1).reshape(128, H * KT)
            * f32(s)
        )
    for nm, w in (("w21", W21), ("w22", W22)):
        p[nm] = np.ascontiguousarray(
            np.clip(
                np.asarray(w, f32)
                .reshape(H, KT, 128, H1)
                .transpose(0, 2, 1, 3)
                .reshape(H, 128, KT * H1)
                * f32(SW),
                -240.0,
                240.0,
            )
        ).astype(fp8)
    for nm, w in (("w31", W31), ("w32", W32)):
        p[nm] = np.ascontiguousarray(
            np.asarray(w, f32)
            .reshape(H, KT, 128, C)
            .transpose(2, 0, 1, 3)
            .reshape(128, H * KT * C)
        ).astype(bf)
    p["b31r"] = np.ascontiguousarray(
        np.broadcast_to(np.asarray(b31, f32).reshape(1, H * C), (128, H * C))
    )
    p["b32r"] = np.ascontiguousarray(
        np.broadcast_to(np.asarray(b32, f32).reshape(1, H * C), (128, H * C))
    )
    w = np.asarray(wt, np.float64)
    e = np.exp(w - w.max())
    p["wsm"] = np.ascontiguousarray(
        np.broadcast_to((e / e.sum()).astype(f32), (128, H))
    )
    return p


def _prep_core(x_shard, xd_shard):
    bf = ml_dtypes.bfloat16
    Bc = x_shard.shape[0]
    Bc_ = x_shard.shape[0]
    xt4 = np.zeros((128, Bc_), np.float32)
    for g in range(4):
        xt4[32 * g : 32 * g + F_IN] = x_shard.T
    return {
        "xt": np.ascontiguousarray(xt4).astype(bf),
        "xbm": np.ascontiguousarray(
            xd_shard.reshape(Bc // 128, 128, F_IN).transpose(1, 0, 2).reshape(128, -1)
        ),
    }


def _gather_out(U, Bc):
    return np.ascontiguousarray(
        U.reshape(128, Bc // 128, C).transpose(1, 0, 2).reshape(Bc, C)
    )


def kernel(x, W1, b1, W21, b21, W22, b22, W31, b31, W32, b32, wt, mean, std,
           sgn=None, itr=None, **_unused):
    x = np.asarray(x, np.float32)
    B = x.shape[0]
    assert B % N_CORES == 0
    Bc = B // N_CORES
    xd = x * np.asarray(std, np.float32) + np.asarray(mean, np.float32)

    shared = _prep_shared(W1, b1, W21, b21, W22, b22, W31, b31, W32, b32, wt)
    in_maps = []
    for i in range(N_CORES):
        rows = slice(i * Bc, (i + 1) * Bc)
        m = dict(shared)
        m.update(_prep_core(x[rows], xd[rows]))
        in_maps.append(m)

    nc = build_abnet(Bc)
    res = run_bass_kernel_spmd(nc, in_maps, core_ids=list(range(N_CORES)))
    out = np.concatenate(
        [_gather_out(np.asarray(res.results[i]["out"], np.float32), Bc)
         for i in range(N_CORES)],
        axis=0,
    )
    return out



# revision 33
# speedup vs baseline: 1.8889x; 1.0685x over previous
"""Trainium2 Bass kernel for nn_ABNet: 10-head MLP ensemble + per-sample QP.

Reference computation (per sample, all heads):
  h1  = relu(x @ W1[h] + b1[h])            x:[B,4]  -> [B,1024]
  x21 = relu(h1 @ W21[h] + b21[h])         -> [B,1024]
  x22 = relu(h1 @ W22[h] + b22[h])         -> [B,1024]
  x31 = x21 @ W31[h] + b31[h]              -> [B,2]
  x32 = 4*sigmoid(x22 @ W32[h] + b32[h])   -> [B,2]
  + closed-form single-constraint QP epilogue, softmax(wt) ensemble.

Strategy: pure data parallel over batch across 8 NeuronCores (B=32768 ->
4096/core).  Feature-major layout on chip (hidden dim on partitions, batch
on the free axis) so all matmuls use natural weight layouts with no
transposes.  The dominant layer-2 matmuls (h1 @ W21 / h1 @ W22, 88% of PE
cycles) run in fp8 e4m3 with MatmulPerfMode.DoubleRow (2 k-tiles per
instruction, 2x PE throughput); h1 is evicted straight to fp8 and W2x is
host-quantized with a x32 scale folded into the bias / epilogue constants.
The QP epilogue runs batch-major after a PE transpose of the tiny per-head
outputs.  Everything is built with the Tile framework (auto scheduling).
"""

import numpy as np
import ml_dtypes

import concourse.bass as bass
import concourse.mybir as mybir
import concourse.tile as tile
from concourse.vector_clock import ScopedClock
from concourse.masks import make_identity
from concourse.bass_utils import run_bass_kernel_spmd

BF16 = mybir.dt.bfloat16
F32 = mybir.dt.float32
FP8 = mybir.dt.float8e4
AF = mybir.ActivationFunctionType
OP = mybir.AluOpType
DR = mybir.MatmulPerfMode.DoubleRow

H, F_IN, H1, C = 10, 4, 1024, 2
KT = H1 // 128  # 8 k-tiles of the hidden dim
OBS_X, OBS_Y, RAD = 40.0, 15.0, 6.0
N_CORES = 8
B_FULL = 32768
SW = 32.0  # fp8 weight scale for W21/W22 (folded into b2x and the epilogue)

_drain_patched = False


def _patch_tile_drain():
    """This container's walrus rejects >2 sync waits on one CTRL op; move the
    Tile kernel-tail drain waits onto individual SP NOPs."""
    global _drain_patched
    if _drain_patched:
        return
    _drain_patched = True

    def _drain_and_barrier(self, tick_clock, wait_clock):
        nc = self.nc
        carrier = nc.sync.nop()
        wait_clock.add_sem_waits(
            carrier.ins, ScopedClock({None: tick_clock.global_clock})
        )
        si = carrier.ins.sync_info
        waits = list(si.on_wait) if si and si.on_wait else []
        if len(waits) > 1:
            carrier.ins.sync_info = mybir.SyncInfo(on_wait=[waits[0]], on_update=[])
            for w in waits[1:]:
                nop = nc.sync.nop()
                nop.ins.sync_info = mybir.SyncInfo(on_wait=[w], on_update=[])
        nc.sync.drain()
        nc.all_engine_barrier()
        assert self.sems is not None
        popped = nc._tile_sem_poison_stack.pop()
        assert popped is self._sem_poison
        nc.clear_and_free_semaphores(list(self.sems.allocated().values()))
        nc.all_engine_barrier()

    tile.TileContext._drain_and_barrier = _drain_and_barrier


def _split_excess_waits(nc, max_waits=1):
    """This walrus build rejects instructions carrying more than a couple of
    semaphore waits; hoist the excess onto same-engine NoOps just before."""
    for fn in nc.m.functions:
        for bb in fn.blocks:
            out = []
            changed = False
            for inst in bb.instructions:
                si = inst.sync_info
                if si is not None and si.on_wait and len(si.on_wait) > max_waits:
                    waits = list(si.on_wait)
                    excess, keep = waits[:-max_waits], waits[-max_waits:]
                    for i in range(0, len(excess), max_waits):
                        nop = mybir.InstNoOp(
                            name=nc.get_next_instruction_name(),
                            engine=inst.engine,
                            ins=[],
                            outs=[],
                            sync_info=mybir.SyncInfo(
                                on_wait=excess[i : i + max_waits], on_update=[]
                            ),
                        )
                        nc.register_instruction(nop)
                        out.append(nop)
                    inst.sync_info = mybir.SyncInfo(
                        on_wait=keep, on_update=list(si.on_update or [])
                    )
                    changed = True
                out.append(inst)
            if changed:
                bb.instructions = out


def build_abnet(Bc: int, W: int = 512, loop_n: int = 1, skip_epilogue: bool = False, pack_l3: bool = False):
    """Build the per-core Bass graph.  Bc = per-core batch, W = batch chunk
    (free-dim width of the big matmuls, <=512 for f32 PSUM)."""
    assert Bc % 128 == 0 and Bc % W == 0
    NB = Bc // W       # batch chunks
    NC_COL = Bc // 128  # batch-major columns
    NW = W // 128      # batch-major columns per chunk
    _patch_tile_drain()

    nc = bass.Bass("TRN2")
    # ---- DRAM parameters (host-prepped layouts) ----
    d_xt = nc.dram_tensor("xt", [128, Bc], BF16, kind="ExternalInput")
    d_xbm = nc.dram_tensor("xbm", [128, NC_COL * F_IN], F32, kind="ExternalInput")
    d_w1 = nc.dram_tensor("w1", [128, H * H1], BF16, kind="ExternalInput")
    d_b1 = nc.dram_tensor("b1", [128, H * KT], F32, kind="ExternalInput")
    d_w21 = nc.dram_tensor("w21", [H, 128, KT * H1], FP8, kind="ExternalInput")
    d_w22 = nc.dram_tensor("w22", [H, 128, KT * H1], FP8, kind="ExternalInput")
    d_b21 = nc.dram_tensor("b21", [128, H * KT], F32, kind="ExternalInput")
    d_b22 = nc.dram_tensor("b22", [128, H * KT], F32, kind="ExternalInput")
    d_w31 = nc.dram_tensor("w31", [128, H * KT * C], BF16, kind="ExternalInput")
    d_w32 = nc.dram_tensor("w32", [128, H * KT * C], BF16, kind="ExternalInput")
    d_b31r = nc.dram_tensor("b31r", [128, H * C], F32, kind="ExternalInput")
    d_b32r = nc.dram_tensor("b32r", [128, H * C], F32, kind="ExternalInput")
    d_wsm = nc.dram_tensor("wsm", [128, H], F32, kind="ExternalInput")
    d_out = nc.dram_tensor("out", [128, NC_COL * C], F32, kind="ExternalOutput")

    from contextlib import ExitStack

    with tile.TileContext(nc) as tc, ExitStack() as ctx:
        const = ctx.enter_context(tc.tile_pool(name="const", bufs=1))
        w2_pool = ctx.enter_context(tc.tile_pool(name="w2", bufs=2))
        h1_pool = ctx.enter_context(tc.tile_pool(name="h1", bufs=4))
        x2_pool = ctx.enter_context(tc.tile_pool(name="x2", bufs=4))
        stage_pool = ctx.enter_context(tc.tile_pool(name="stage", bufs=1))
        ps1 = ctx.enter_context(tc.tile_pool(name="ps1", bufs=2, space="PSUM"))
        ps2 = ctx.enter_context(tc.tile_pool(name="ps2", bufs=4, space="PSUM"))
        ps3 = ctx.enter_context(tc.tile_pool(name="ps3", bufs=2, space="PSUM"))
        ep_pool = ctx.enter_context(tc.tile_pool(name="ep", bufs=1))
        epb_pool = ctx.enter_context(tc.tile_pool(name="epb", bufs=1))
        tmp_pool = ctx.enter_context(tc.tile_pool(name="tmp", bufs=12))
        acc_pool = ctx.enter_context(tc.tile_pool(name="acc", bufs=4))

        # ---- constant / small loads ----
        xt = const.tile([128, Bc], BF16, tag="xt")
        nc.sync.dma_start(xt[:], d_xt[:])
        xbm = const.tile([128, NC_COL * F_IN], F32, tag="xbm")
        nc.sync.dma_start(xbm[:], d_xbm[:])
        w1 = const.tile([128, H * H1], BF16, tag="w1")
        nc.sync.dma_start(w1[:], d_w1[:])
        b1 = const.tile([128, H * KT], F32, tag="b1")
        nc.sync.dma_start(b1[:], d_b1[:])
        b21 = const.tile([128, H * KT], F32, tag="b21")
        nc.sync.dma_start(b21[:], d_b21[:])
        b22 = const.tile([128, H * KT], F32, tag="b22")
        nc.sync.dma_start(b22[:], d_b22[:])
        w31 = const.tile([128, H * KT * C], BF16, tag="w31")
        nc.sync.dma_start(w31[:], d_w31[:])
        w32 = const.tile([128, H * KT * C], BF16, tag="w32")
        nc.sync.dma_start(w32[:], d_w32[:])
        b31r = const.tile([128, H * C], F32, tag="b31r")
        nc.sync.dma_start(b31r[:], d_b31r[:])
        b32r = const.tile([128, H * C], F32, tag="b32r")
        nc.sync.dma_start(b32r[:], d_b32r[:])
        wsm = const.tile([128, H], F32, tag="wsm")
        nc.sync.dma_start(wsm[:], d_wsm[:])
        ident = const.tile([128, 128], F32, tag="ident")
        make_identity(nc, ident)

        def body(_iv=None):
            # staging for per-head QP inputs, feature-major: rows 4h..4h+3 =
            # [z1, z2, s32_c0, s32_c1] of head h (z = -(x21@W31 + b31))
            S = stage_pool.tile([4 * H, Bc], F32, tag="S")

            # ---- main loop: heads x batch chunks.  Layer-1 matmuls for the
            # NEXT chunk are interleaved one at a time between the current
            # chunk's layer-2 out-tile groups so the in-order PE queue never
            # stalls on an L1 PSUM eviction; evictions are pinned per engine
            # (L2 -> ACT, L1 -> DVE, L3 -> Pool) to keep queues conflict-free.
            def emit_l1_tile(h, bc, h1, t):
                bsl = bass.ds(bc * W, W)
                # operands at partition base 32*(t%4): bass auto-derives
                # tile_position so K=4 matmuls pack onto independent
                # 32-row groups of the PE array
                base = 32 * (t % 4)
                p1t = ps1.tile([128, W], F32, tag="ps1", name="p1t")
                nc.tensor.matmul(
                    p1t[:],
                    w1[base : base + F_IN, h * H1 + t * 128 : h * H1 + (t + 1) * 128],
                    xt[base : base + F_IN, bsl],
                    start=True,
                    stop=True,
                    tile_position=(base, 0),
                )
                # relu+bias eviction straight to fp8 on DVE
                nc.vector.tensor_scalar(
                    h1[:, t, :],
                    p1t[:],
                    b1[:, h * KT + t : h * KT + t + 1],
                    0.0,
                    OP.add,
                    OP.max,
                )

            # Batch chunks are processed in PAIRS: for each (out-tile, k-pair)
            # the stationary W2 slice is loaded once and streamed against both
            # chunks' h1 (second matmul carries ldweights=False).  With the
            # broken ldw-opt pass in this walrus build, weight loads are not
            # pipelined with streams, so halving the load count saves ~107ns
            # per DoubleRow matmul pair.
            NG = NB // 2  # chunk pairs
            h1_tiles = {}
            for h in range(H):
                w21 = w2_pool.tile([128, KT, H1], FP8, tag="w21", name="w21")
                w22 = w2_pool.tile([128, KT, H1], FP8, tag="w22", name="w22")
                for k in range(KT):
                    nc.sync.dma_start(
                        w21[:, k, :], d_w21[h, :, k * H1 : (k + 1) * H1]
                    )
                    nc.sync.dma_start(
                        w22[:, k, :], d_w22[h, :, k * H1 : (k + 1) * H1]
                    )
                for bg in range(NG):
                    bcs = (2 * bg, 2 * bg + 1)
                    if (h, bcs[0]) not in h1_tiles:
                        # startup only: emit the first pair's L1 as a burst
                        for bc in bcs:
                            h1_tiles[(h, bc)] = h1_pool.tile(
                                [128, KT, W], FP8, tag="h1", name="h1"
                            )
                            for t in range(KT):
                                emit_l1_tile(h, bc, h1_tiles[(h, bc)], t)
                    nh, nbg = (h, bg + 1) if bg + 1 < NG else (h + 1, 0)
                    nxts = None
                    if nh < H:
                        nxts = [
                            h1_pool.tile([128, KT, W], FP8, tag="h1", name="h1")
                            for _ in range(2)
                        ]
                        h1_tiles[(nh, 2 * nbg)] = nxts[0]
                        h1_tiles[(nh, 2 * nbg + 1)] = nxts[1]
                    h1s = [h1_tiles.pop((h, bc)) for bc in bcs]
                    bsls = [bass.ds(bc * W, W) for bc in bcs]
                    # -- layers 2+3 for each branch, both chunks of the pair
                    for m, (w2, b2, w3, srow) in enumerate(
                        (
                            (w21, b21, w31, 4 * h),
                            (w22, b22, w32, 4 * h + 2),
                        )
                    ):
                        x2s = [
                            x2_pool.tile([128, KT, W], BF16, tag="x2", name="x2")
                            for _ in range(2)
                        ]
                        for t in range(KT):
                            p2ts = [
                                ps2.tile([128, W], F32, tag="ps2", name="p2t")
                                for _ in range(2)
                            ]
                            for j in range(KT // 2):
                                # fp8 DoubleRow: 2 k-tiles per instruction;
                                # one weight load serves both chunks
                                for i in range(2):
                                    mm = nc.tensor.matmul(
                                        p2ts[i][:],
                                        w2[:, 2 * j : 2 * j + 2, t * 128 : (t + 1) * 128],
                                        h1s[i][:, 2 * j : 2 * j + 2, :],
                                        start=(j == 0),
                                        stop=(j == KT // 2 - 1),
                                        perf_mode=DR,
                                    )
                                    if i == 1:
                                        mm.ins.ldweights = False
                            # PSUM evictions pinned to ACT
                            for i in range(2):
                                nc.scalar.activation(
                                    x2s[i][:, t, :],
                                    p2ts[i][:],
                                    AF.Relu,
                                    bias=b2[:, h * KT + t : h * KT + t + 1],
                                )
                            # one next-pair L1 matmul between out-tile groups
                            if nxts is not None:
                                emit_l1_tile(
                                    nh, 2 * nbg + m, nxts[m], t
                                )
                        for i in range(2):
                            p3 = ps3.tile([128, W], F32, tag="ps3", name="p3")
                            for t in range(KT):
                                nc.tensor.matmul(
                                    p3[:C, :],
                                    w3[:, (h * KT + t) * C : (h * KT + t + 1) * C],
                                    x2s[i][:, t, :],
                                    start=(t == 0),
                                    stop=(t == KT - 1),
                                )
                            # stage raw accumulators (bias in the epilogue);
                            # engines cannot write partition offsets that are
                            # not multiples of 32: bounce + DMA into S
                            bnc = acc_pool.tile([C, W], F32, tag="bnc", name="bnc")
                            nc.vector.tensor_copy(bnc[:], p3[:C, :])
                            nc.sync.dma_start(S[srow : srow + C, bsls[i]], bnc[:])

            if skip_epilogue:
                nc.sync.dma_start(d_out[: 4 * H, :], S[:, : NC_COL * C])
                return
            # ---- transpose staging to batch-major: ST[p, c*40+r] = S[r, c*128+p]
            R = 4 * H
            ST = stage_pool.tile([128, NC_COL * R], F32, tag="ST")
            for c in range(NC_COL):
                # reuse the ps2 PSUM tag (main loop is done) to stay in budget
                pt = ps2.tile([128, W], F32, tag="ps2", name="pt")
                nc.tensor.transpose(
                    pt[:, :R], S[:, c * 128 : (c + 1) * 128], ident[:R, :R]
                )
                nc.vector.tensor_copy(ST[:, c * R : (c + 1) * R], pt[:, :R])

            # q(j) = [128, H, NC_COL] strided view of quantity j for all heads
            STr = ST.rearrange("p (c g j) -> p j g c", g=H, j=4)
            xbm3 = xbm.rearrange("p (c f) -> p f c", f=F_IN)
            NCH = H * NC_COL

            def q(j):
                return STr[:, j]

            def ep(tag, pool=ep_pool):
                return pool.tile([128, NC_COL], F32, tag=tag, name=tag)

            def tmp():
                return tmp_pool.tile([128, NC_COL], F32, tag="tmp", name="tmp")

            def big(tag):
                tl = epb_pool.tile([128, NCH], F32, tag=tag, name=tag)
                return tl, tl.rearrange("p (g c) -> p g c", g=H)

            def bc3(t):
                # [128, NC_COL] -> [128, H, NC_COL] head-broadcast
                return t.unsqueeze(1).broadcast_to((128, H, NC_COL))

            # ---- geometry (batch-major, denormalized positions from host) ----
            px, py, th, v = (xbm3[:, f, :] for f in range(4))
            st_, ct_, dx, dy = ep("st"), ep("ct"), ep("dx"), ep("dy")
            PI = float(np.pi)

            def wrap_to_pi(dst, src, folds=3):
                # dst = src - 2*pi*k in [-pi, pi]; handles |src| <= (2*folds+1)*pi
                c = tmp()
                nc.vector.tensor_scalar(c[:], src, PI, None, OP.is_gt)
                nc.vector.scalar_tensor_tensor(dst[:], c[:], -2 * PI, src, OP.mult, OP.add)
                for _ in range(folds):
                    c = tmp()
                    nc.vector.tensor_scalar(c[:], dst[:], -PI, None, OP.is_lt)
                    nc.vector.scalar_tensor_tensor(dst[:], c[:], 2 * PI, dst[:], OP.mult, OP.add)
                    c = tmp()
                    nc.vector.tensor_scalar(c[:], dst[:], PI, None, OP.is_gt)
                    nc.vector.scalar_tensor_tensor(dst[:], c[:], -2 * PI, dst[:], OP.mult, OP.add)

            thr = ep("thr")
            wrap_to_pi(thr, th)
            nc.scalar.activation(st_[:], thr[:], AF.Sin)
            nc.vector.tensor_scalar_add(thr[:], th, PI / 2)
            wrap_to_pi(thr, thr[:])
            nc.scalar.activation(ct_[:], thr[:], AF.Sin)
            nc.vector.tensor_scalar_add(dx[:], px, -OBS_X)
            nc.vector.tensor_scalar_add(dy[:], py, -OBS_Y)
            vst2, vct2 = ep("vst2"), ep("vct2")
            t0 = tmp()
            nc.vector.tensor_mul(t0[:], v, st_[:])
            nc.vector.tensor_scalar_mul(vst2[:], t0[:], 2.0)
            t0 = tmp()
            nc.vector.tensor_mul(t0[:], v, ct_[:])
            nc.vector.tensor_scalar_mul(vct2[:], t0[:], 2.0)
            barrier, bdot, lf2b = ep("barrier"), ep("bdot"), ep("lf2b")
            ta, tb = tmp(), tmp()
            nc.vector.tensor_mul(ta[:], dx[:], dx[:])
            nc.vector.tensor_mul(tb[:], dy[:], dy[:])
            nc.vector.scalar_tensor_tensor(
                barrier[:], ta[:], -(RAD * RAD), tb[:], OP.add, OP.add
            )
            ta, tb = tmp(), tmp()
            nc.vector.tensor_mul(ta[:], dx[:], vct2[:])
            nc.vector.tensor_mul(tb[:], dy[:], vst2[:])
            nc.vector.tensor_add(bdot[:], ta[:], tb[:])
            ta = tmp()
            nc.vector.tensor_mul(ta[:], v, v)
            nc.vector.tensor_scalar_mul(lf2b[:], ta[:], 2.0)
            G1, G2, invgg = ep("G1"), ep("G2"), ep("invgg")
            ta, tb = tmp(), tmp()
            nc.vector.tensor_mul(ta[:], dx[:], vst2[:])
            nc.vector.tensor_mul(tb[:], dy[:], vct2[:])
            nc.vector.tensor_sub(G1[:], ta[:], tb[:])
            ta, tb = tmp(), tmp()
            nc.vector.tensor_mul(ta[:], dx[:], ct_[:])
            nc.vector.tensor_mul(tb[:], dy[:], st_[:])
            nc.vector.tensor_add(ta[:], ta[:], tb[:])
            nc.vector.tensor_scalar_mul(G2[:], ta[:], -2.0)
            ta, tb = tmp(), tmp()
            nc.vector.tensor_mul(ta[:], G1[:], G1[:])
            nc.vector.tensor_mul(tb[:], G2[:], G2[:])
            nc.vector.scalar_tensor_tensor(ta[:], ta[:], 1e-12, tb[:], OP.add, OP.add)
            nc.vector.reciprocal(invgg[:], ta[:])

            # ---- QP for all heads at once on [128, H*NC_COL] tiles ----
            b31c0 = b31r[:, 0 : 2 * H : 2].to_broadcast((128, H, NC_COL))
            b31c1 = b31r[:, 1 : 2 * H : 2].to_broadcast((128, H, NC_COL))
            b32c0 = b32r[:, 0 : 2 * H : 2].to_broadcast((128, H, NC_COL))
            w_all = wsm[:, 0:H].to_broadcast((128, H, NC_COL))

            # z = -(s31/SW + b31)  (s31 carries the x32 fp8 weight scale)
            z1t, z1a = big("z1a")
            nc.vector.scalar_tensor_tensor(z1a, q(0), -1.0 / SW, b31c0, OP.mult, OP.subtract)
            z2t, z2a = big("z2a")
            nc.vector.scalar_tensor_tensor(z2a, q(1), -1.0 / SW, b31c1, OP.mult, OP.subtract)
            # a = 4*sigmoid(s32_c0/SW + b32_c0) for all heads
            at, aa = big("aa")
            nc.vector.scalar_tensor_tensor(aa, q(2), 1.0 / SW, b32c0, OP.mult, OP.add)
            nc.scalar.activation(at[:], at[:], AF.Sigmoid)
            nc.vector.tensor_scalar_mul(at[:], at[:], 4.0)
            # head 0: p1 = a[g=0]; then a[g=0] is replaced by col-1 sigmoid
            p1v = ep("p1v")
            nc.vector.tensor_copy(p1v[:], at[:, 0:NC_COL])
            h0a = tmp()
            nc.vector.tensor_scalar(h0a[:], STr[:, 3, 0, :], 1.0 / SW, b32r[:, 1:2], OP.mult, OP.add)
            nc.scalar.activation(h0a[:], h0a[:], AF.Sigmoid)
            nc.vector.tensor_scalar_mul(at[:, 0:NC_COL], h0a[:], 4.0)
            p1b = bc3(p1v)

            # h_qp = lf2b + (p1+a)*bdot + p1*a*barrier
            smt, sm3 = big("smt")
            mut, mu3 = big("mut")
            nc.vector.tensor_tensor(sm3, p1b, aa, OP.add)
            nc.vector.tensor_tensor(mu3, p1b, aa, OP.mult)
            nc.vector.tensor_tensor(sm3, sm3, bc3(bdot), OP.mult)
            nc.vector.tensor_tensor(mu3, mu3, bc3(barrier), OP.mult)
            nc.vector.tensor_add(smt[:], smt[:], mut[:])
            nc.vector.tensor_tensor(sm3, sm3, bc3(lf2b), OP.add)
            # gz = G1*z1 + G2*z2 ; lam = relu(gz - hqp) * invgg
            gat, ga3 = big("gat")
            nc.vector.tensor_tensor(ga3, bc3(G1), z1a, OP.mult)
            nc.vector.tensor_tensor(mu3, bc3(G2), z2a, OP.mult)
            nc.vector.tensor_add(gat[:], gat[:], mut[:])
            nc.vector.tensor_sub(gat[:], gat[:], smt[:])
            nc.vector.tensor_relu(gat[:], gat[:])
            nc.vector.tensor_tensor(ga3, ga3, bc3(invgg), OP.mult)  # lam
            # weighted sums over heads (w_all applied, then pairwise tree)
            nc.vector.tensor_tensor(z1a, z1a, w_all, OP.mult)
            nc.vector.tensor_tensor(z2a, z2a, w_all, OP.mult)
            nc.vector.tensor_tensor(ga3, ga3, w_all, OP.mult)
            NC2 = NC_COL

            def tree_sum(t):
                nc.vector.tensor_add(t[:, 0 : 5 * NC2], t[:, 0 : 5 * NC2], t[:, 5 * NC2 : 10 * NC2])
                nc.vector.tensor_add(t[:, 0:NC2], t[:, 0:NC2], t[:, 4 * NC2 : 5 * NC2])
                nc.vector.tensor_add(t[:, 0 : 2 * NC2], t[:, 0 : 2 * NC2], t[:, 2 * NC2 : 4 * NC2])
                nc.vector.tensor_add(t[:, 0:NC2], t[:, 0:NC2], t[:, NC2 : 2 * NC2])

            tree_sum(z1t)
            tree_sum(z2t)
            tree_sum(gat)

            # ---- u = acc_z - acc_lam * G ; write interleaved [128, (c,2)] ----
            U = stage_pool.tile([128, NC_COL * C], F32, tag="U")
            U3 = U.rearrange("p (c ch) -> p ch c", ch=C)
            ta = tmp()
            nc.vector.tensor_mul(ta[:], gat[:, 0:NC_COL], G1[:])
            nc.vector.tensor_sub(U3[:, 0, :], z1t[:, 0:NC_COL], ta[:])
            tb = tmp()
            nc.vector.tensor_mul(tb[:], gat[:, 0:NC_COL], G2[:])
            nc.vector.tensor_sub(U3[:, 1, :], z2t[:, 0:NC_COL], tb[:])
            nc.sync.dma_start(d_out[:], U[:])

        if loop_n == 1:
            body()
        else:
            with tc.For_i(0, loop_n, 1) as _iv:
                body(_iv)


    _split_excess_waits(nc)
    return nc


# ---------------- host-side preparation ----------------


def _prep_shared(W1, b1, W21, b21, W22, b22, W31, b31, W32, b32, wt):
    bf = ml_dtypes.bfloat16
    f32 = np.float32
    p = {}
    w1flat = np.asarray(W1, f32).transpose(1, 0, 2).reshape(F_IN, H * H1)
    w14 = np.zeros((128, H * H1), f32)
    for g in range(4):
        w14[32 * g : 32 * g + F_IN] = w1flat
    p["w1"] = np.ascontiguousarray(w14).astype(bf)
    fp8 = ml_dtypes.float8_e4m3
    for nm, b, s in (("b1", b1, 1.0), ("b21", b21, SW), ("b22", b22, SW)):
        p[nm] = np.ascontiguousarray(
            np.asarray(b, f32).reshape(H, KT, 128).transpose(2, 0, 1).reshape(128, H * KT)
            * f32(s)
        )
    for nm, w in (("w21", W21), ("w22", W22)):
        p[nm] = np.ascontiguousarray(
            np.clip(
                np.asarray(w, f32)
                .reshape(H, KT, 128, H1)
                .transpose(0, 2, 1, 3)
                .reshape(H, 128, KT * H1)
                * f32(SW),
                -240.0,
                240.0,
            )
        ).astype(fp8)
    for nm, w in (("w31", W31), ("w32", W32)):
        p[nm] = np.ascontiguousarray(
            np.asarray(w, f32)
            .reshape(H, KT, 128, C)
            .transpose(2, 0, 1, 3)
            .reshape(128, H * KT * C)
        ).astype(bf)
    p["b31r"] = np.ascontiguousarray(
        np.broadcast_to(np.asarray(b31, f32).reshape(1, H * C), (128, H * C))
    )
    p["b32r"] = np.ascontiguousarray(
        np.broadcast_to(np.asarray(b32, f32).reshape(1, H * C), (128, H * C))
    )
    w = np.asarray(wt, np.float64)
    e = np.exp(w - w.max())
    p["wsm"] = np.ascontiguousarray(
        np.broadcast_to((e / e.sum()).astype(f32), (128, H))
    )
    return p


def _prep_core(x_shard, xd_shard):
    bf = ml_dtypes.bfloat16
    Bc = x_shard.shape[0]
    Bc_ = x_shard.shape[0]
    xt4 = np.zeros((128, Bc_), np.float32)
    for g in range(4):
        xt4[32 * g : 32 * g + F_IN] = x_shard.T
    return {
        "xt": np.ascontiguousarray(xt4).astype(bf),
        "xbm": np.ascontiguousarray(
            xd_shard.reshape(Bc // 128, 128, F_IN).transpose(1, 0, 2).reshape(128, -1)
        ),
    }


def _gather_out(U, Bc):
    return np.ascontiguousarray(
        U.reshape(128, Bc // 128, C).transpose(1, 0, 2).reshape(Bc, C)
    )


def kernel(x, W1, b1, W21, b21, W22, b22, W31, b31, W32, b32, wt, mean, std,
           sgn=None, itr=None, **_unused):
    x = np.asarray(x, np.float32)
    B = x.shape[0]
    assert B % N_CORES == 0
    Bc = B // N_CORES
    xd = x * np.asarray(std, np.float32) + np.asarray(mean, np.float32)

    shared = _prep_shared(W1, b1, W21, b21, W22, b22, W31, b31, W32, b32, wt)
    in_maps = []
    for i in range(N_CORES):
        rows = slice(i * Bc, (i + 1) * Bc)
        m = dict(shared)
        m.update(_prep_core(x[rows], xd[rows]))
        in_maps.append(m)

    nc = build_abnet(Bc)
    res = run_bass_kernel_spmd(nc, in_maps, core_ids=list(range(N_CORES)))
    out = np.concatenate(
        [_gather_out(np.asarray(res.results[i]["out"], np.float32), Bc)
         for i in range(N_CORES)],
        axis=0,
    )
    return out

